# revision 98
# baseline (speedup 1.0000x reference)
"""GQA kernel for Trainium2, 8 NeuronCores.

Problem: x[2,2048,2048] -> GQA(16 heads, 4 kv groups, dk=128) -> out[2,2048,2048]

Sharding: core c handles (batch b = c//4, kv-group g = c%4): the 4 query heads
of one group on one batch. Host sums the 4 per-group partial outputs per batch
(row-parallel O-proj reduction) and adds bo.

Numerics / engine split:
  - Q/K/V projections run as error-compensated fp8e4m3 DoubleRow matmuls: the
    host splits x (*16) and W (*64) into hi+lo fp8 pairs (scaled to dodge
    fp8's subnormal floor); pass 1 contracts hi*hi over chunk pairs, pass 2
    puts (w_hi*x_lo + w_lo*x_hi) in the two DoubleRow slots of one
    instruction.  0.75 cycles per 128-deep contraction vs 1.0 for bf16, at
    ~bf16 accuracy.
  - O-projection uses the same compensated-fp8 DoubleRow trick (aot split
    into hi/lo on DVE, Wo split on host).
  - scores stay bf16.  attn*V: heads 0-1 of each group use fp8 attn probs
    (exp emitted as fp8e4m3, scaled by 1/2 to fit the 240 max) against
    hi/lo-compensated fp8 V via DoubleRow (2x); heads 2-3 stay bf16.  This
    splits the fp8-attn quantization error in half: measured 1.44e-2 L2
    vs the 2e-2 budget.
  - ACT engine does the softmax Exp (PSUM f32 -> SBUF bf16/fp8).
  - DVE does bias-add copies, softmax normalize, transpose copies, output
    scaling; the final-block drain alternates output copies onto ACT.
  - Emission is software-pipelined: each attention unit (q-block, head)
    interleaves its 16 score matmuls with AV/Qproj/Oproj filler work so the
    PE never stalls waiting for ACT; the projection pre-phase is paced to
    the serialized DMA arrival order of x chunks and weight slices.
"""

import math
from collections import deque

import numpy as np
import ml_dtypes

import concourse.bass as bass
import concourse.mybir as mybir
import concourse.tile as tile
from concourse import bacc
from concourse.bass_utils import run_bass_kernel_spmd
from concourse.masks import make_identity

F32 = mybir.dt.float32
BF16 = mybir.dt.bfloat16
F8 = mybir.dt.float8e4

D = 2048          # d_model
S = 2048          # seq len
DK = 128          # head dim
HPG = 4           # heads per kv group
QCOLS = HPG * DK  # 512 q columns per core
N_CORES = 8
SCALE = 1.0 / math.sqrt(DK)

ND = D // 128     # 16 contraction chunks for projections
NSK = S // 128    # 16 key chunks
SQB = 512         # q-block (scores psum free size)
NJ = S // SQB     # 4 q-blocks
NSUB = SQB // 128  # 4 128-row subtiles per q-block
DR = mybir.MatmulPerfMode.DoubleRow
X_SCALE = 16.0   # fp8 hi/lo split scales (avoid subnormal underflow of lo)
W_SCALE = 64.0
INV_PROJ_SCALE = 1.0 / (X_SCALE * W_SCALE)
AO_SCALE = 16.0  # attn-out fp8 scale (via 1/16 ones column)
WO_SCALE = 64.0  # Wo fp8 hi/lo scale
INV_O_SCALE = 1.0 / (AO_SCALE * WO_SCALE)
FP8_HEADS = (0, 1)       # heads whose attn probs are fp8 (DoubleRow AV)
ATTN8_BIAS = math.log(0.5)  # exp scaled by 0.5 so max ~147 fits fp8e4m3


def build_program(debug_dumps=False):
    nc = bacc.Bacc("TRN2", target_bir_lowering=False, debug=False,
                   num_devices=N_CORES)

    # fp8 hi/lo pair layouts; slot order: x=(lo,hi), w=(hi,lo) so that
    #   pass1: lhsT=w[:,2t:2t+2,0,:]  rhs=x[:,2t:2t+2,1,:]  -> hi*hi pairs
    #   pass2: lhsT=w[:,c,:,:]        rhs=x[:,c,:,:]        -> hi*lo + lo*hi
    xmix = nc.dram_tensor("xmix", [128, ND, 2, S], F8, kind="ExternalInput").ap()
    wqmix = nc.dram_tensor("wqmix", [128, ND, 2, QCOLS], F8, kind="ExternalInput").ap()
    # K/V weights slot-major: innermost run is ND*DK = 2KB per (partition,
    # slot), so the startup-critical transfers avoid the small-descriptor
    # latency penalty (1.45us each instead of 2.9us).
    wkmix = nc.dram_tensor("wkmix", [128, 2, ND, DK], F8, kind="ExternalInput").ap()
    wvmix = nc.dram_tensor("wvmix", [128, 2, ND, DK], F8, kind="ExternalInput").ap()
    wo = nc.dram_tensor("wo", [128, HPG, 2, D], F8, kind="ExternalInput").ap()
    bq = nc.dram_tensor("bq", [128, HPG], F32, kind="ExternalInput").ap()
    bk = nc.dram_tensor("bk", [128, 1], F32, kind="ExternalInput").ap()
    bv = nc.dram_tensor("bv", [128, 1], F32, kind="ExternalInput").ap()
    out = nc.dram_tensor("out", [S, D], F32, kind="ExternalOutput").ap()
    if debug_dumps:
        dbg_kt = nc.dram_tensor("dbg_kt", [128, S], BF16, kind="ExternalOutput").ap()
        dbg_qt = nc.dram_tensor("dbg_qt", [128, HPG, S], BF16, kind="ExternalOutput").ap()
        dbg_vones = nc.dram_tensor("dbg_vones", [128, NSK, 132], BF16, kind="ExternalOutput").ap()
        dbg_attn = nc.dram_tensor("dbg_attn", [128, SQB], BF16, kind="ExternalOutput").ap()
        dbg_aot = nc.dram_tensor("dbg_aot", [128, HPG, 2, SQB], F8, kind="ExternalOutput").ap()

    with tile.TileContext(nc) as tc:
        with (
            tc.tile_pool(name="singles", bufs=1) as singles,
            tc.tile_pool(name="attn", bufs=26) as attnpool,
            tc.tile_pool(name="aot", bufs=2) as aotpool,
            tc.tile_pool(name="ao", bufs=4) as aopool,
            tc.tile_pool(name="osb", bufs=4) as outpool,
            tc.tile_pool(name="small", bufs=6) as smallpool,
            tc.tile_pool(name="psSC", bufs=3, space="PSUM") as psSC,
            tc.tile_pool(name="psPJ", bufs=1, space="PSUM") as psPJ,
            tc.tile_pool(name="psPO", bufs=2, space="PSUM") as psPO,
            tc.tile_pool(name="psAV", bufs=2, space="PSUM") as psAV,
        ):
            # ---- resident inputs, ordered to pace the projection pre-phase
            # (DMA engines are serial: deliver exactly what the PE needs
            # next) ----
            wk_sb = singles.tile([128, 2, ND, DK], F8)
            nc.sync.dma_start(out=wk_sb, in_=wkmix)
            bk_sb = singles.tile([128, 1], F32)
            nc.sync.dma_start(out=bk_sb, in_=bk)
            x_sb = singles.tile([128, ND, 2, S], F8)
            nc.sync.dma_start(out=x_sb[:, 0:4, :, 0:SQB],
                              in_=xmix[:, 0:4, :, 0:SQB])
            nc.sync.dma_start(out=x_sb[:, 4:8, :, 0:SQB],
                              in_=xmix[:, 4:8, :, 0:SQB])
            wv_sb = singles.tile([128, 2, ND, DK], F8)
            nc.sync.dma_start(out=wv_sb, in_=wvmix)
            bv_sb = singles.tile([128, 1], F32)
            nc.sync.dma_start(out=bv_sb, in_=bv)
            nc.sync.dma_start(out=x_sb[:, 8:12, :, 0:SQB],
                              in_=xmix[:, 8:12, :, 0:SQB])
            nc.sync.dma_start(out=x_sb[:, 12:16, :, 0:SQB],
                              in_=xmix[:, 12:16, :, 0:SQB])
            nc.sync.dma_start(out=x_sb[:, 0:8, :, SQB:2 * SQB],
                              in_=xmix[:, 0:8, :, SQB:2 * SQB])
            nc.sync.dma_start(out=x_sb[:, 8:16, :, SQB:2 * SQB],
                              in_=xmix[:, 8:16, :, SQB:2 * SQB])
            wq_sb = singles.tile([128, ND, 2, QCOLS], F8)
            bq_sb = singles.tile([128, HPG], F32)
            nc.sync.dma_start(out=wq_sb[:, :, :, 0:128], in_=wqmix[:, :, :, 0:128])
            nc.sync.dma_start(out=bq_sb, in_=bq)
            nc.sync.dma_start(out=x_sb[:, 0:8, :, 2 * SQB:3 * SQB],
                              in_=xmix[:, 0:8, :, 2 * SQB:3 * SQB])
            nc.sync.dma_start(out=x_sb[:, 8:16, :, 2 * SQB:3 * SQB],
                              in_=xmix[:, 8:16, :, 2 * SQB:3 * SQB])
            nc.sync.dma_start(out=wq_sb[:, :, :, 128:256], in_=wqmix[:, :, :, 128:256])
            nc.sync.dma_start(out=x_sb[:, 0:8, :, 3 * SQB:4 * SQB],
                              in_=xmix[:, 0:8, :, 3 * SQB:4 * SQB])
            nc.sync.dma_start(out=x_sb[:, 8:16, :, 3 * SQB:4 * SQB],
                              in_=xmix[:, 8:16, :, 3 * SQB:4 * SQB])
            nc.sync.dma_start(out=wq_sb[:, :, :, 256:384], in_=wqmix[:, :, :, 256:384])
            nc.sync.dma_start(out=wq_sb[:, :, :, 384:512], in_=wqmix[:, :, :, 384:512])
            wo_sb = singles.tile([128, HPG, 2, D], F8)
            nc.sync.dma_start(out=wo_sb, in_=wo)

            ident16 = singles.tile([128, 128], BF16)
            make_identity(nc, ident16)
            ln_half = singles.tile([128, 1], F32)
            nc.vector.memset(ln_half, ATTN8_BIAS)

            qt_sb = singles.tile([128, HPG, S], BF16)    # QT per head [dk, S]
            kt_sb = singles.tile([128, S], BF16)         # KT [dk, S]
            vt_sb = singles.tile([128, S], BF16)         # VT [dk, S]
            vones = singles.tile([128, NSK, 132], BF16)  # [V | 1/16] per key chunk
            nc.vector.memset(vones[:, :, 128:129], 1.0 / AO_SCALE)
            # fp8 hi/lo copies of [V | 1/16] for the DoubleRow AV path
            vones8h = singles.tile([128, NSK, 132], F8)
            nc.vector.memset(vones8h[:, :, 128:129], 1.0 / AO_SCALE)
            vones8l = singles.tile([128, NSK, 132], F8)
            nc.vector.memset(vones8l[:, :, 128:129], 0.0)

            # ---- helper emitters ----
            def comp_proj(w_ap, sl, psum, slot_major=False):
                """Accumulate compensated-fp8 projection of x block sl into
                psum: per chunk pair, hi*hi over the pair then the two cross
                terms (chunk demand is monotonic, so the first pair can start
                as soon as the first half of an x chunk-block lands)."""
                n = ND // 2
                for t in range(n):
                    lhsT = (w_ap[:, 0, 2 * t:2 * t + 2, :] if slot_major
                            else w_ap[:, 2 * t:2 * t + 2, 0, :])
                    nc.tensor.matmul(
                        psum, lhsT=lhsT,
                        rhs=x_sb[:, 2 * t:2 * t + 2, 1, sl],
                        start=(t == 0), stop=False, perf_mode=DR)
                    for c in (2 * t, 2 * t + 1):
                        lhsT = (w_ap[:, :, c, :] if slot_major
                                else w_ap[:, c, :, :])
                        nc.tensor.matmul(
                            psum, lhsT=lhsT,
                            rhs=x_sb[:, c, :, sl],
                            start=False, stop=(c == ND - 1), perf_mode=DR)

            def kv_proj_block(jb, which, pool=None):
                sl = bass.ts(jb, SQB)
                pool = pool or psPJ
                p = pool.tile([128, SQB], F32,
                              tag=("po" if pool is psPO else "pj"),
                              name=f"pj{jb}{which}")
                if which == "k":
                    comp_proj(wk_sb, sl, p, slot_major=True)
                    nc.vector.tensor_scalar(kt_sb[:, sl], p, INV_PROJ_SCALE,
                                            bk_sb, op0=mybir.AluOpType.mult,
                                            op1=mybir.AluOpType.add)
                else:
                    comp_proj(wv_sb, sl, p, slot_major=True)
                    nc.vector.tensor_scalar(vt_sb[:, sl], p, INV_PROJ_SCALE,
                                            bv_sb, op0=mybir.AluOpType.mult,
                                            op1=mybir.AluOpType.add)

            def q_proj_head(j, h, pool=None):
                sl = bass.ts(j, SQB)
                pool = pool or psPJ
                p = pool.tile([128, SQB], F32,
                              tag=("po" if pool is psPO else "pj"),
                              name=f"q{j}{h}")
                w = wq_sb[:, :, :, bass.ts(h, 128)]
                comp_proj(w, sl, p)
                nc.vector.tensor_scalar(qt_sb[:, h, sl], p, INV_PROJ_SCALE,
                                        bq_sb[:, h:h + 1],
                                        op0=mybir.AluOpType.mult,
                                        op1=mybir.AluOpType.add)

            def v_transpose(sk):
                pt = psAV.tile([128, 128], BF16, tag="av")
                nc.tensor.transpose(pt, vt_sb[:, bass.ts(sk, 128)], ident16)
                nc.vector.tensor_copy(vones[:, sk, 0:128], pt)
                hi = vones8h[:, sk, 0:128]
                nc.vector.tensor_copy(hi, pt)
                nc.vector.tensor_tensor(vones8l[:, sk, 0:128], pt, hi,
                                        mybir.AluOpType.subtract)

            attn_tiles = {}
            aot_tiles = {}

            def av_sub(j, h, sub):
                """attn@[V|1/16] for 128 q rows, then normalize (scaled by 16)
                + transpose + fp8 hi/lo split into aot_tiles[j]."""
                mode8, tiles = attn_tiles[(j, h)]
                pav = psAV.tile([128, 132], F32, tag="av")
                if mode8:
                    ssl = bass.ts(sub, 128)
                    for t in range(NSK // 2):
                        nc.tensor.matmul(
                            pav[:, 0:129], lhsT=tiles[t][:, :, ssl],
                            rhs=vones8h[:, 2 * t:2 * t + 2, 0:129],
                            start=(t == 0), stop=False, perf_mode=DR)
                    for t in range(NSK // 2):
                        nc.tensor.matmul(
                            pav[:, 0:129], lhsT=tiles[t][:, :, ssl],
                            rhs=vones8l[:, 2 * t:2 * t + 2, 0:129],
                            start=False, stop=(t == NSK // 2 - 1),
                            perf_mode=DR)
                else:
                    for sk in range(NSK):
                        nc.tensor.matmul(
                            pav[:, 0:129],
                            lhsT=tiles[sk][:, bass.ts(sub, 128)],
                            rhs=vones[:, sk, 0:129],
                            start=(sk == 0), stop=(sk == NSK - 1))
                recip = smallpool.tile([128, 1], F32)
                nc.vector.reciprocal(recip, pav[:, 128:129])
                ao = aopool.tile([128, 128], BF16)
                nc.vector.tensor_scalar_mul(ao, pav[:, 0:128], recip)
                pt = psAV.tile([128, 128], BF16, tag="av")
                nc.tensor.transpose(pt, ao, ident16)
                aot = aot_tiles[j]
                hi = aot[:, h, 1, bass.ts(sub, 128)]
                nc.vector.tensor_copy(hi, pt)
                nc.vector.tensor_tensor(
                    aot[:, h, 0, bass.ts(sub, 128)], pt, hi,
                    mybir.AluOpType.subtract)

            def oproj_sub(j, sub, dcs, copy_engine="dve", width=512):
                """Compensated-fp8 O-projection for q rows [j*SQB + sub*128,
                +128), d_model chunks dcs (each `width` wide)."""
                aot = aot_tiles[j]
                for dc in dcs:
                    po = psPO.tile([128, width], F32, tag="po", name="po")
                    dsl = bass.ts(dc, width)
                    ssl = bass.ts(sub, 128)
                    for t in range(HPG // 2):
                        nc.tensor.matmul(
                            po, lhsT=aot[:, 2 * t:2 * t + 2, 1, ssl],
                            rhs=wo_sb[:, 2 * t:2 * t + 2, 0, dsl],
                            start=(t == 0), stop=False, perf_mode=DR)
                    for h in range(HPG):
                        nc.tensor.matmul(
                            po, lhsT=aot[:, h, :, ssl],
                            rhs=wo_sb[:, h, :, dsl],
                            start=False, stop=(h == HPG - 1), perf_mode=DR)
                    osb = outpool.tile([128, width], F32, tag="osb", name="osb")
                    eng = copy_engine
                    if eng == "mix":
                        eng = "act" if dc % 2 == 0 else "dve"
                    if eng == "act":
                        nc.scalar.activation(
                            out=osb, in_=po,
                            func=mybir.ActivationFunctionType.Copy,
                            scale=INV_O_SCALE)
                    else:
                        nc.vector.tensor_scalar_mul(osb, po, INV_O_SCALE)
                    nc.sync.dma_start(
                        out=out[j * SQB + sub * 128: j * SQB + (sub + 1) * 128,
                                dsl],
                        in_=osb)

            def emit_scores_chunk(j, h, sk, tiles, mode8, u):
                ps = psSC.tile([128, SQB], F32, tag="sc", name=f"sc{u}_{sk}")
                nc.tensor.matmul(ps, lhsT=kt_sb[:, bass.ts(sk, 128)],
                                 rhs=qt_sb[:, h, bass.ts(j, SQB)],
                                 start=True, stop=True)
                if mode8:
                    if sk % 2 == 0:
                        a8 = attnpool.tile([128, 2, SQB], F8, tag="a8",
                                           bufs=18, name=f"a8_{u}_{sk}")
                        tiles.append(a8)
                    nc.scalar.activation(
                        out=tiles[-1][:, sk % 2, :], in_=ps,
                        func=mybir.ActivationFunctionType.Exp,
                        scale=SCALE, bias=ln_half)
                else:
                    a = attnpool.tile([128, SQB], BF16, tag="a", bufs=30,
                                      name=f"a_{u}_{sk}")
                    nc.scalar.activation(
                        out=a, in_=ps,
                        func=mybir.ActivationFunctionType.Exp, scale=SCALE)
                    tiles.append(a)

            # ---- pre-phase: all projections for block 0 + K/V, paced to the
            # DMA arrival order of the x chunks and wq head slices; scores of
            # unit (0,0) are woven in as the K blocks they need complete, so
            # the ACT engine starts its softmax work ~20us earlier.
            # Alternate psum pools so the next block's matmuls don't wait on
            # the previous block's PSUM->SBUF copy (psPO is idle until the
            # first O-projection, well after this phase).
            pre = [("kv", 0, "k"), ("kv", 0, "v"), ("vt", 0, None),
                   ("kv", 1, "k"), ("kv", 1, "v"), ("vt", 1, None),
                   ("q", 0, 0),
                   ("kv", 2, "k"), ("kv", 2, "v"), ("vt", 2, None),
                   ("q", 0, 1),
                   ("kv", 3, "k"), ("kv", 3, "v"), ("vt", 3, None),
                   ("q", 0, 2), ("q", 0, 3)]
            flip = 0
            for kind, a, b in pre:
                if kind == "kv":
                    kv_proj_block(a, b, pool=(psPO if flip % 2 else psPJ))
                    flip += 1
                elif kind == "q":
                    q_proj_head(a, b, pool=(psPO if flip % 2 else psPJ))
                    flip += 1
                else:
                    for sk in range(4 * a, 4 * a + 4):
                        v_transpose(sk)

            # ---- attention units, software pipelined ----
            units = [(j, h) for j in range(NJ) for h in range(HPG)]

            def unit_fillers(u):
                """PE filler closures for unit u (consumed between score
                matmuls)."""
                j, h = units[u]
                fill = []
                if u >= 1:
                    pj, ph = units[u - 1]
                    avs = [lambda pj=pj, ph=ph, sub=sub: av_sub(pj, ph, sub)
                           for sub in range(NSUB)]
                    ops = []
                    if j >= 1:
                        # O-projection for block j-1, sub h (split in two);
                        # independent of this unit's ACT work, so lead with it
                        # when aot(j-1) is already complete (h > 0).
                        ops = [lambda j=j, h=h: oproj_sub(j - 1, h, range(0, 2)),
                               lambda j=j, h=h: oproj_sub(j - 1, h, range(2, 4))]
                    if h == 0:
                        fill = avs + ops
                    else:
                        fill = [ops[0], avs[0], avs[1], ops[1],
                                avs[2], avs[3]] if ops else avs
                    if u >= 2:
                        del attn_tiles[units[u - 2]]
                # Q projection four units ahead (block 0 is in the pre-phase).
                if u + 4 < len(units):
                    nj, nh = units[u + 4]
                    fill.append(lambda nj=nj, nh=nh: q_proj_head(nj, nh))
                return deque(fill)

            for u, (j, h) in enumerate(units):
                if h == 0:
                    aot_j = aotpool.tile([128, HPG, 2, SQB], F8, tag="aot",
                                         name=f"aot{j}")
                    aot_tiles[j] = aot_j
                fill = unit_fillers(u)
                mode8 = h in FP8_HEADS
                tiles = []
                for sk in range(NSK):
                    emit_scores_chunk(j, h, sk, tiles, mode8, u)
                    if sk % 2 == 1 and fill:
                        fill.popleft()()
                while fill:
                    fill.popleft()()
                attn_tiles[(j, h)] = (mode8, tiles)

            # ---- drain: last unit's AV + O-projection of last block,
            # software-pipelined (AV one sub ahead of its O-projection) with
            # output copies on the otherwise-idle ACT engine ----
            av_sub(NJ - 1, HPG - 1, 0)
            for sub in range(NSUB):
                if sub + 1 < NSUB:
                    av_sub(NJ - 1, HPG - 1, sub + 1)
                oproj_sub(NJ - 1, sub, range(0, 4), copy_engine="mix")

            if debug_dumps:
                nc.sync.dma_start(out=dbg_kt, in_=kt_sb)
                nc.sync.dma_start(out=dbg_qt, in_=qt_sb)
                nc.sync.dma_start(out=dbg_vones, in_=vones)
                nc.sync.dma_start(out=dbg_attn, in_=attn_tiles[(NJ - 1, HPG - 1)][1][0])
                nc.sync.dma_start(out=dbg_aot, in_=aot_tiles[NJ - 1])

    nc.compile()
    return nc


_NC_CACHE = None


def _get_program():
    global _NC_CACHE
    if _NC_CACHE is None:
        _NC_CACHE = build_program()
    return _NC_CACHE


def _hi_lo(a):
    """Split float32 array into fp8e4m3 hi + lo parts."""
    hi = np.asarray(a, dtype=ml_dtypes.float8_e4m3)
    lo = np.asarray(a - hi.astype(np.float32), dtype=ml_dtypes.float8_e4m3)
    return hi, lo


def _mix(a, ncols, slot_order, scale):
    """[D, ncols] f32 -> [128, ND, 2, ncols] fp8 with given hi/lo slot order."""
    a3 = np.ascontiguousarray(a.reshape(ND, 128, ncols)) * np.float32(scale)
    hi, lo = _hi_lo(a3)
    parts = {"hi": hi, "lo": lo}
    stacked = np.stack([parts[slot_order[0]], parts[slot_order[1]]], axis=2)
    return np.ascontiguousarray(stacked.transpose(1, 0, 2, 3))


def _mix_kv(wk_g, wv_g):
    """Two [D, DK] f32 -> [128, ND, 2, 2, DK] fp8 (slot order hi,lo; then
    k/v)."""
    k = _mix(wk_g, DK, ("hi", "lo"), W_SCALE)
    v = _mix(wv_g, DK, ("hi", "lo"), W_SCALE)
    return np.ascontiguousarray(np.stack([k, v], axis=3))


def kernel(x, Wq, bq, Wk, bk, Wv, bv, Wo, bo):
    x = np.asarray(x, np.float32)
    Wq = np.asarray(Wq, np.float32)
    Wk = np.asarray(Wk, np.float32)
    Wv = np.asarray(Wv, np.float32)
    Wo = np.asarray(Wo, np.float32)
    nc = _get_program()

    xmix = [_mix(np.ascontiguousarray(x[b].T), S, ("lo", "hi"), X_SCALE)
            for b in range(x.shape[0])]

    in_maps = []
    for c in range(N_CORES):
        b, g = divmod(c, HPG)
        wo_g = Wo[g * QCOLS:(g + 1) * QCOLS, :].reshape(HPG, 128, D) * np.float32(WO_SCALE)
        wo_hi, wo_lo = _hi_lo(wo_g)
        wo_mix = np.stack([wo_hi, wo_lo], axis=2)  # [HPG, 128, 2, D]
        in_maps.append({
            "xmix": xmix[b],
            "wqmix": _mix(Wq[:, g * QCOLS:(g + 1) * QCOLS], QCOLS, ("hi", "lo"), W_SCALE),
            "wkmix": np.ascontiguousarray(
                _mix(Wk[:, g * DK:(g + 1) * DK], DK, ("hi", "lo"), W_SCALE)
                .transpose(0, 2, 1, 3)),
            "wvmix": np.ascontiguousarray(
                _mix(Wv[:, g * DK:(g + 1) * DK], DK, ("hi", "lo"), W_SCALE)
                .transpose(0, 2, 1, 3)),
            "wo": np.ascontiguousarray(wo_mix.transpose(1, 0, 2, 3)),
            "bq": np.ascontiguousarray(
                np.asarray(bq, np.float32)[g * QCOLS:(g + 1) * QCOLS]
                .reshape(HPG, 128).T),
            "bk": np.ascontiguousarray(
                np.asarray(bk, np.float32)[g * DK:(g + 1) * DK].reshape(128, 1)),
            "bv": np.ascontiguousarray(
                np.asarray(bv, np.float32)[g * DK:(g + 1) * DK].reshape(128, 1)),
        })

    res = run_bass_kernel_spmd(nc, in_maps, core_ids=list(range(N_CORES))).results

    outv = np.zeros((x.shape[0], S, D), np.float32)
    for c in range(N_CORES):
        b = c // HPG
        outv[b] += res[c]["out"]
    outv += np.asarray(bo, np.float32)
    return outv


# revision 99
# speedup vs baseline: 1.0178x; 1.0178x over previous
"""GQA kernel for Trainium2, 8 NeuronCores.

Problem: x[2,2048,2048] -> GQA(16 heads, 4 kv groups, dk=128) -> out[2,2048,2048]

Sharding: core c handles (batch b = c//4, kv-group g = c%4): the 4 query heads
of one group on one batch. Host sums the 4 per-group partial outputs per batch
(row-parallel O-proj reduction) and adds bo.

Numerics / engine split:
  - Q/K/V projections run as error-compensated fp8e4m3 DoubleRow matmuls: the
    host splits x (*16) and W (*64) into hi+lo fp8 pairs (scaled to dodge
    fp8's subnormal floor); pass 1 contracts hi*hi over chunk pairs, pass 2
    puts (w_hi*x_lo + w_lo*x_hi) in the two DoubleRow slots of one
    instruction.  0.75 cycles per 128-deep contraction vs 1.0 for bf16, at
    ~bf16 accuracy.
  - O-projection uses the same compensated-fp8 DoubleRow trick (aot split
    into hi/lo on DVE, Wo split on host).
  - scores stay bf16.  attn*V: heads 0-1 of each group use fp8 attn probs
    (exp emitted as fp8e4m3, scaled by 1/2 to fit the 240 max) against
    hi/lo-compensated fp8 V via DoubleRow (2x); heads 2-3 stay bf16.  This
    splits the fp8-attn quantization error in half: measured 1.44e-2 L2
    vs the 2e-2 budget.
  - ACT engine does the softmax Exp (PSUM f32 -> SBUF bf16/fp8).
  - DVE does bias-add copies, softmax normalize, transpose copies, output
    scaling; the final-block drain alternates output copies onto ACT.
  - Emission is software-pipelined: each attention unit (q-block, head)
    interleaves its 16 score matmuls with AV/Qproj/Oproj filler work so the
    PE never stalls waiting for ACT; the projection pre-phase is paced to
    the serialized DMA arrival order of x chunks and weight slices.
"""

import math
from collections import deque

import numpy as np
import ml_dtypes

import concourse.bass as bass
import concourse.mybir as mybir
import concourse.tile as tile
from concourse import bacc
from concourse.bass_utils import run_bass_kernel_spmd
from concourse.masks import make_identity

F32 = mybir.dt.float32
BF16 = mybir.dt.bfloat16
F8 = mybir.dt.float8e4

D = 2048          # d_model
S = 2048          # seq len
DK = 128          # head dim
HPG = 4           # heads per kv group
QCOLS = HPG * DK  # 512 q columns per core
N_CORES = 8
SCALE = 1.0 / math.sqrt(DK)

ND = D // 128     # 16 contraction chunks for projections
NSK = S // 128    # 16 key chunks
SQB = 512         # q-block (scores psum free size)
NJ = S // SQB     # 4 q-blocks
NSUB = SQB // 128  # 4 128-row subtiles per q-block
DR = mybir.MatmulPerfMode.DoubleRow
X_SCALE = 16.0   # fp8 hi/lo split scales (avoid subnormal underflow of lo)
W_SCALE = 64.0
INV_PROJ_SCALE = 1.0 / (X_SCALE * W_SCALE)
AO_SCALE = 16.0  # attn-out fp8 scale (via 1/16 ones column)
WO_SCALE = 64.0  # Wo fp8 hi/lo scale
INV_O_SCALE = 1.0 / (AO_SCALE * WO_SCALE)
FP8_HEADS = (0, 1, 2)    # heads whose attn probs are fp8 (DoubleRow AV)
ATTN8_BIAS = math.log(0.5)  # exp scaled by 0.5 so max ~147 fits fp8e4m3


def build_program(debug_dumps=False):
    nc = bacc.Bacc("TRN2", target_bir_lowering=False, debug=False,
                   num_devices=N_CORES)

    # fp8 hi/lo pair layouts; slot order: x=(lo,hi), w=(hi,lo) so that
    #   pass1: lhsT=w[:,2t:2t+2,0,:]  rhs=x[:,2t:2t+2,1,:]  -> hi*hi pairs
    #   pass2: lhsT=w[:,c,:,:]        rhs=x[:,c,:,:]        -> hi*lo + lo*hi
    xmix = nc.dram_tensor("xmix", [128, ND, 2, S], F8, kind="ExternalInput").ap()
    wqmix = nc.dram_tensor("wqmix", [128, ND, 2, QCOLS], F8, kind="ExternalInput").ap()
    # K/V weights slot-major: innermost run is ND*DK = 2KB per (partition,
    # slot), so the startup-critical transfers avoid the small-descriptor
    # latency penalty (1.45us each instead of 2.9us).
    wkmix = nc.dram_tensor("wkmix", [128, 2, ND, DK], F8, kind="ExternalInput").ap()
    wvmix = nc.dram_tensor("wvmix", [128, 2, ND, DK], F8, kind="ExternalInput").ap()
    wo = nc.dram_tensor("wo", [128, HPG, 2, D], F8, kind="ExternalInput").ap()
    bq = nc.dram_tensor("bq", [128, HPG], F32, kind="ExternalInput").ap()
    bk = nc.dram_tensor("bk", [128, 1], F32, kind="ExternalInput").ap()
    bv = nc.dram_tensor("bv", [128, 1], F32, kind="ExternalInput").ap()
    out = nc.dram_tensor("out", [S, D], F32, kind="ExternalOutput").ap()
    if debug_dumps:
        dbg_kt = nc.dram_tensor("dbg_kt", [128, S], BF16, kind="ExternalOutput").ap()
        dbg_qt = nc.dram_tensor("dbg_qt", [128, HPG, S], BF16, kind="ExternalOutput").ap()
        dbg_vones = nc.dram_tensor("dbg_vones", [128, NSK, 132], BF16, kind="ExternalOutput").ap()
        dbg_attn = nc.dram_tensor("dbg_attn", [128, SQB], BF16, kind="ExternalOutput").ap()
        dbg_aot = nc.dram_tensor("dbg_aot", [128, HPG, 2, SQB], F8, kind="ExternalOutput").ap()

    with tile.TileContext(nc) as tc:
        with (
            tc.tile_pool(name="singles", bufs=1) as singles,
            tc.tile_pool(name="attn", bufs=26) as attnpool,
            tc.tile_pool(name="aot", bufs=2) as aotpool,
            tc.tile_pool(name="ao", bufs=4) as aopool,
            tc.tile_pool(name="osb", bufs=4) as outpool,
            tc.tile_pool(name="small", bufs=6) as smallpool,
            tc.tile_pool(name="psSC", bufs=3, space="PSUM") as psSC,
            tc.tile_pool(name="psPJ", bufs=1, space="PSUM") as psPJ,
            tc.tile_pool(name="psPO", bufs=2, space="PSUM") as psPO,
            tc.tile_pool(name="psAV", bufs=2, space="PSUM") as psAV,
        ):
            # ---- resident inputs, ordered to pace the projection pre-phase
            # (DMA engines are serial: deliver exactly what the PE needs
            # next) ----
            wk_sb = singles.tile([128, 2, ND, DK], F8)
            nc.sync.dma_start(out=wk_sb, in_=wkmix)
            bk_sb = singles.tile([128, 1], F32)
            nc.sync.dma_start(out=bk_sb, in_=bk)
            x_sb = singles.tile([128, ND, 2, S], F8)
            nc.sync.dma_start(out=x_sb[:, 0:4, :, 0:SQB],
                              in_=xmix[:, 0:4, :, 0:SQB])
            nc.sync.dma_start(out=x_sb[:, 4:8, :, 0:SQB],
                              in_=xmix[:, 4:8, :, 0:SQB])
            wv_sb = singles.tile([128, 2, ND, DK], F8)
            nc.sync.dma_start(out=wv_sb, in_=wvmix)
            bv_sb = singles.tile([128, 1], F32)
            nc.sync.dma_start(out=bv_sb, in_=bv)
            nc.sync.dma_start(out=x_sb[:, 8:12, :, 0:SQB],
                              in_=xmix[:, 8:12, :, 0:SQB])
            nc.sync.dma_start(out=x_sb[:, 12:16, :, 0:SQB],
                              in_=xmix[:, 12:16, :, 0:SQB])
            nc.sync.dma_start(out=x_sb[:, 0:8, :, SQB:2 * SQB],
                              in_=xmix[:, 0:8, :, SQB:2 * SQB])
            nc.sync.dma_start(out=x_sb[:, 8:16, :, SQB:2 * SQB],
                              in_=xmix[:, 8:16, :, SQB:2 * SQB])
            wq_sb = singles.tile([128, ND, 2, QCOLS], F8)
            bq_sb = singles.tile([128, HPG], F32)
            nc.sync.dma_start(out=wq_sb[:, :, :, 0:128], in_=wqmix[:, :, :, 0:128])
            nc.sync.dma_start(out=bq_sb, in_=bq)
            nc.sync.dma_start(out=x_sb[:, 0:8, :, 2 * SQB:3 * SQB],
                              in_=xmix[:, 0:8, :, 2 * SQB:3 * SQB])
            nc.sync.dma_start(out=x_sb[:, 8:16, :, 2 * SQB:3 * SQB],
                              in_=xmix[:, 8:16, :, 2 * SQB:3 * SQB])
            nc.sync.dma_start(out=wq_sb[:, :, :, 128:256], in_=wqmix[:, :, :, 128:256])
            nc.sync.dma_start(out=x_sb[:, 0:8, :, 3 * SQB:4 * SQB],
                              in_=xmix[:, 0:8, :, 3 * SQB:4 * SQB])
            nc.sync.dma_start(out=x_sb[:, 8:16, :, 3 * SQB:4 * SQB],
                              in_=xmix[:, 8:16, :, 3 * SQB:4 * SQB])
            nc.sync.dma_start(out=wq_sb[:, :, :, 256:384], in_=wqmix[:, :, :, 256:384])
            nc.sync.dma_start(out=wq_sb[:, :, :, 384:512], in_=wqmix[:, :, :, 384:512])
            wo_sb = singles.tile([128, HPG, 2, D], F8)
            nc.sync.dma_start(out=wo_sb, in_=wo)

            ident16 = singles.tile([128, 128], BF16)
            make_identity(nc, ident16)
            ln_half = singles.tile([128, 1], F32)
            nc.vector.memset(ln_half, ATTN8_BIAS)

            qt_sb = singles.tile([128, HPG, S], BF16)    # QT per head [dk, S]
            kt_sb = singles.tile([128, S], BF16)         # KT [dk, S]
            vt_sb = singles.tile([128, S], BF16)         # VT [dk, S]
            vones = singles.tile([128, NSK, 132], BF16)  # [V | 1/16] per key chunk
            nc.vector.memset(vones[:, :, 128:129], 1.0 / AO_SCALE)
            # fp8 hi/lo copies of [V | 1/16] for the DoubleRow AV path
            vones8h = singles.tile([128, NSK, 132], F8)
            nc.vector.memset(vones8h[:, :, 128:129], 1.0 / AO_SCALE)
            vones8l = singles.tile([128, NSK, 132], F8)
            nc.vector.memset(vones8l[:, :, 128:129], 0.0)

            # ---- helper emitters ----
            def comp_proj(w_ap, sl, psum, slot_major=False):
                """Accumulate compensated-fp8 projection of x block sl into
                psum: per chunk pair, hi*hi over the pair then the two cross
                terms (chunk demand is monotonic, so the first pair can start
                as soon as the first half of an x chunk-block lands)."""
                n = ND // 2
                for t in range(n):
                    lhsT = (w_ap[:, 0, 2 * t:2 * t + 2, :] if slot_major
                            else w_ap[:, 2 * t:2 * t + 2, 0, :])
                    nc.tensor.matmul(
                        psum, lhsT=lhsT,
                        rhs=x_sb[:, 2 * t:2 * t + 2, 1, sl],
                        start=(t == 0), stop=False, perf_mode=DR)
                    for c in (2 * t, 2 * t + 1):
                        lhsT = (w_ap[:, :, c, :] if slot_major
                                else w_ap[:, c, :, :])
                        nc.tensor.matmul(
                            psum, lhsT=lhsT,
                            rhs=x_sb[:, c, :, sl],
                            start=False, stop=(c == ND - 1), perf_mode=DR)

            def kv_proj_block(jb, which, pool=None):
                sl = bass.ts(jb, SQB)
                pool = pool or psPJ
                p = pool.tile([128, SQB], F32,
                              tag=("po" if pool is psPO else "pj"),
                              name=f"pj{jb}{which}")
                if which == "k":
                    comp_proj(wk_sb, sl, p, slot_major=True)
                    nc.vector.tensor_scalar(kt_sb[:, sl], p, INV_PROJ_SCALE,
                                            bk_sb, op0=mybir.AluOpType.mult,
                                            op1=mybir.AluOpType.add)
                else:
                    comp_proj(wv_sb, sl, p, slot_major=True)
                    nc.vector.tensor_scalar(vt_sb[:, sl], p, INV_PROJ_SCALE,
                                            bv_sb, op0=mybir.AluOpType.mult,
                                            op1=mybir.AluOpType.add)

            def q_proj_head(j, h, pool=None):
                sl = bass.ts(j, SQB)
                pool = pool or psPJ
                p = pool.tile([128, SQB], F32,
                              tag=("po" if pool is psPO else "pj"),
                              name=f"q{j}{h}")
                w = wq_sb[:, :, :, bass.ts(h, 128)]
                comp_proj(w, sl, p)
                nc.vector.tensor_scalar(qt_sb[:, h, sl], p, INV_PROJ_SCALE,
                                        bq_sb[:, h:h + 1],
                                        op0=mybir.AluOpType.mult,
                                        op1=mybir.AluOpType.add)

            def v_transpose(sk):
                pt = psAV.tile([128, 128], BF16, tag="av")
                nc.tensor.transpose(pt, vt_sb[:, bass.ts(sk, 128)], ident16)
                nc.vector.tensor_copy(vones[:, sk, 0:128], pt)
                hi = vones8h[:, sk, 0:128]
                nc.vector.tensor_copy(hi, pt)
                nc.vector.tensor_tensor(vones8l[:, sk, 0:128], pt, hi,
                                        mybir.AluOpType.subtract)

            attn_tiles = {}
            aot_tiles = {}

            def av_sub(j, h, sub):
                """attn@[V|1/16] for 128 q rows, then normalize (scaled by 16)
                + transpose + fp8 hi/lo split into aot_tiles[j]."""
                mode8, tiles = attn_tiles[(j, h)]
                pav = psAV.tile([128, 132], F32, tag="av")
                if mode8:
                    ssl = bass.ts(sub, 128)
                    for t in range(NSK // 2):
                        nc.tensor.matmul(
                            pav[:, 0:129], lhsT=tiles[t][:, :, ssl],
                            rhs=vones8h[:, 2 * t:2 * t + 2, 0:129],
                            start=(t == 0), stop=False, perf_mode=DR)
                    for t in range(NSK // 2):
                        nc.tensor.matmul(
                            pav[:, 0:129], lhsT=tiles[t][:, :, ssl],
                            rhs=vones8l[:, 2 * t:2 * t + 2, 0:129],
                            start=False, stop=(t == NSK // 2 - 1),
                            perf_mode=DR)
                else:
                    for sk in range(NSK):
                        nc.tensor.matmul(
                            pav[:, 0:129],
                            lhsT=tiles[sk][:, bass.ts(sub, 128)],
                            rhs=vones[:, sk, 0:129],
                            start=(sk == 0), stop=(sk == NSK - 1))
                recip = smallpool.tile([128, 1], F32)
                nc.vector.reciprocal(recip, pav[:, 128:129])
                ao = aopool.tile([128, 128], BF16)
                nc.vector.tensor_scalar_mul(ao, pav[:, 0:128], recip)
                pt = psAV.tile([128, 128], BF16, tag="av")
                nc.tensor.transpose(pt, ao, ident16)
                aot = aot_tiles[j]
                hi = aot[:, h, 1, bass.ts(sub, 128)]
                nc.vector.tensor_copy(hi, pt)
                nc.vector.tensor_tensor(
                    aot[:, h, 0, bass.ts(sub, 128)], pt, hi,
                    mybir.AluOpType.subtract)

            def oproj_sub(j, sub, dcs, copy_engine="dve", width=512):
                """Compensated-fp8 O-projection for q rows [j*SQB + sub*128,
                +128), d_model chunks dcs (each `width` wide)."""
                aot = aot_tiles[j]
                for dc in dcs:
                    po = psPO.tile([128, width], F32, tag="po", name="po")
                    dsl = bass.ts(dc, width)
                    ssl = bass.ts(sub, 128)
                    for t in range(HPG // 2):
                        nc.tensor.matmul(
                            po, lhsT=aot[:, 2 * t:2 * t + 2, 1, ssl],
                            rhs=wo_sb[:, 2 * t:2 * t + 2, 0, dsl],
                            start=(t == 0), stop=False, perf_mode=DR)
                    for h in range(HPG):
                        nc.tensor.matmul(
                            po, lhsT=aot[:, h, :, ssl],
                            rhs=wo_sb[:, h, :, dsl],
                            start=False, stop=(h == HPG - 1), perf_mode=DR)
                    osb = outpool.tile([128, width], F32, tag="osb", name="osb")
                    eng = copy_engine
                    if eng == "mix":
                        eng = "act" if dc % 2 == 0 else "dve"
                    if eng == "act":
                        nc.scalar.activation(
                            out=osb, in_=po,
                            func=mybir.ActivationFunctionType.Copy,
                            scale=INV_O_SCALE)
                    else:
                        nc.vector.tensor_scalar_mul(osb, po, INV_O_SCALE)
                    nc.sync.dma_start(
                        out=out[j * SQB + sub * 128: j * SQB + (sub + 1) * 128,
                                dsl],
                        in_=osb)

            def emit_scores_chunk(j, h, sk, tiles, mode8, u):
                ps = psSC.tile([128, SQB], F32, tag="sc", name=f"sc{u}_{sk}")
                nc.tensor.matmul(ps, lhsT=kt_sb[:, bass.ts(sk, 128)],
                                 rhs=qt_sb[:, h, bass.ts(j, SQB)],
                                 start=True, stop=True)
                if mode8:
                    if sk % 2 == 0:
                        a8 = attnpool.tile([128, 2, SQB], F8, tag="a8",
                                           bufs=18, name=f"a8_{u}_{sk}")
                        tiles.append(a8)
                    nc.scalar.activation(
                        out=tiles[-1][:, sk % 2, :], in_=ps,
                        func=mybir.ActivationFunctionType.Exp,
                        scale=SCALE, bias=ln_half)
                else:
                    a = attnpool.tile([128, SQB], BF16, tag="a", bufs=30,
                                      name=f"a_{u}_{sk}")
                    nc.scalar.activation(
                        out=a, in_=ps,
                        func=mybir.ActivationFunctionType.Exp, scale=SCALE)
                    tiles.append(a)

            # ---- pre-phase: all projections for block 0 + K/V, paced to the
            # DMA arrival order of the x chunks and wq head slices; scores of
            # unit (0,0) are woven in as the K blocks they need complete, so
            # the ACT engine starts its softmax work ~20us earlier.
            # Alternate psum pools so the next block's matmuls don't wait on
            # the previous block's PSUM->SBUF copy (psPO is idle until the
            # first O-projection, well after this phase).
            pre = [("kv", 0, "k"), ("kv", 0, "v"), ("vt", 0, None),
                   ("kv", 1, "k"), ("kv", 1, "v"), ("vt", 1, None),
                   ("q", 0, 0),
                   ("kv", 2, "k"), ("kv", 2, "v"), ("vt", 2, None),
                   ("q", 0, 1),
                   ("kv", 3, "k"), ("kv", 3, "v"), ("vt", 3, None),
                   ("q", 0, 2), ("q", 0, 3)]
            flip = 0
            for kind, a, b in pre:
                if kind == "kv":
                    kv_proj_block(a, b, pool=(psPO if flip % 2 else psPJ))
                    flip += 1
                elif kind == "q":
                    q_proj_head(a, b, pool=(psPO if flip % 2 else psPJ))
                    flip += 1
                else:
                    for sk in range(4 * a, 4 * a + 4):
                        v_transpose(sk)

            # ---- attention units, software pipelined ----
            units = [(j, h) for j in range(NJ) for h in range(HPG)]

            def unit_fillers(u):
                """PE filler closures for unit u (consumed between score
                matmuls)."""
                j, h = units[u]
                fill = []
                if u >= 1:
                    pj, ph = units[u - 1]
                    avs = [lambda pj=pj, ph=ph, sub=sub: av_sub(pj, ph, sub)
                           for sub in range(NSUB)]
                    ops = []
                    if j >= 1:
                        # O-projection for block j-1, sub h (split in two);
                        # independent of this unit's ACT work, so lead with it
                        # when aot(j-1) is already complete (h > 0).
                        ops = [lambda j=j, h=h: oproj_sub(j - 1, h, range(0, 2)),
                               lambda j=j, h=h: oproj_sub(j - 1, h, range(2, 4))]
                    if h == 0:
                        fill = avs + ops
                    else:
                        fill = [ops[0], avs[0], avs[1], ops[1],
                                avs[2], avs[3]] if ops else avs
                    if u >= 2:
                        del attn_tiles[units[u - 2]]
                # Q projection four units ahead (block 0 is in the pre-phase).
                if u + 4 < len(units):
                    nj, nh = units[u + 4]
                    fill.append(lambda nj=nj, nh=nh: q_proj_head(nj, nh))
                return deque(fill)

            for u, (j, h) in enumerate(units):
                if h == 0:
                    aot_j = aotpool.tile([128, HPG, 2, SQB], F8, tag="aot",
                                         name=f"aot{j}")
                    aot_tiles[j] = aot_j
                fill = unit_fillers(u)
                mode8 = h in FP8_HEADS
                tiles = []
                for sk in range(NSK):
                    emit_scores_chunk(j, h, sk, tiles, mode8, u)
                    if sk % 2 == 1 and fill:
                        fill.popleft()()
                while fill:
                    fill.popleft()()
                attn_tiles[(j, h)] = (mode8, tiles)

            # ---- drain: last unit's AV + O-projection of last block,
            # software-pipelined (AV one sub ahead of its O-projection) with
            # output copies on the otherwise-idle ACT engine ----
            av_sub(NJ - 1, HPG - 1, 0)
            for sub in range(NSUB):
                if sub + 1 < NSUB:
                    av_sub(NJ - 1, HPG - 1, sub + 1)
                oproj_sub(NJ - 1, sub, range(0, 4), copy_engine="mix")

            if debug_dumps:
                nc.sync.dma_start(out=dbg_kt, in_=kt_sb)
                nc.sync.dma_start(out=dbg_qt, in_=qt_sb)
                nc.sync.dma_start(out=dbg_vones, in_=vones)
                nc.sync.dma_start(out=dbg_attn, in_=attn_tiles[(NJ - 1, HPG - 1)][1][0])
                nc.sync.dma_start(out=dbg_aot, in_=aot_tiles[NJ - 1])

    nc.compile()
    return nc


_NC_CACHE = None


def _get_program():
    global _NC_CACHE
    if _NC_CACHE is None:
        _NC_CACHE = build_program()
    return _NC_CACHE


def _hi_lo(a):
    """Split float32 array into fp8e4m3 hi + lo parts."""
    hi = np.asarray(a, dtype=ml_dtypes.float8_e4m3)
    lo = np.asarray(a - hi.astype(np.float32), dtype=ml_dtypes.float8_e4m3)
    return hi, lo


def _mix(a, ncols, slot_order, scale):
    """[D, ncols] f32 -> [128, ND, 2, ncols] fp8 with given hi/lo slot order."""
    a3 = np.ascontiguousarray(a.reshape(ND, 128, ncols)) * np.float32(scale)
    hi, lo = _hi_lo(a3)
    parts = {"hi": hi, "lo": lo}
    stacked = np.stack([parts[slot_order[0]], parts[slot_order[1]]], axis=2)
    return np.ascontiguousarray(stacked.transpose(1, 0, 2, 3))


def _mix_kv(wk_g, wv_g):
    """Two [D, DK] f32 -> [128, ND, 2, 2, DK] fp8 (slot order hi,lo; then
    k/v)."""
    k = _mix(wk_g, DK, ("hi", "lo"), W_SCALE)
    v = _mix(wv_g, DK, ("hi", "lo"), W_SCALE)
    return np.ascontiguousarray(np.stack([k, v], axis=3))


def kernel(x, Wq, bq, Wk, bk, Wv, bv, Wo, bo):
    x = np.asarray(x, np.float32)
    Wq = np.asarray(Wq, np.float32)
    Wk = np.asarray(Wk, np.float32)
    Wv = np.asarray(Wv, np.float32)
    Wo = np.asarray(Wo, np.float32)
    nc = _get_program()

    xmix = [_mix(np.ascontiguousarray(x[b].T), S, ("lo", "hi"), X_SCALE)
            for b in range(x.shape[0])]

    in_maps = []
    for c in range(N_CORES):
        b, g = divmod(c, HPG)
        wo_g = Wo[g * QCOLS:(g + 1) * QCOLS, :].reshape(HPG, 128, D) * np.float32(WO_SCALE)
        wo_hi, wo_lo = _hi_lo(wo_g)
        wo_mix = np.stack([wo_hi, wo_lo], axis=2)  # [HPG, 128, 2, D]
        in_maps.append({
            "xmix": xmix[b],
            "wqmix": _mix(Wq[:, g * QCOLS:(g + 1) * QCOLS], QCOLS, ("hi", "lo"), W_SCALE),
            "wkmix": np.ascontiguousarray(
                _mix(Wk[:, g * DK:(g + 1) * DK], DK, ("hi", "lo"), W_SCALE)
                .transpose(0, 2, 1, 3)),
            "wvmix": np.ascontiguousarray(
                _mix(Wv[:, g * DK:(g + 1) * DK], DK, ("hi", "lo"), W_SCALE)
                .transpose(0, 2, 1, 3)),
            "wo": np.ascontiguousarray(wo_mix.transpose(1, 0, 2, 3)),
            "bq": np.ascontiguousarray(
                np.asarray(bq, np.float32)[g * QCOLS:(g + 1) * QCOLS]
                .reshape(HPG, 128).T),
            "bk": np.ascontiguousarray(
                np.asarray(bk, np.float32)[g * DK:(g + 1) * DK].reshape(128, 1)),
            "bv": np.ascontiguousarray(
                np.asarray(bv, np.float32)[g * DK:(g + 1) * DK].reshape(128, 1)),
        })

    res = run_bass_kernel_spmd(nc, in_maps, core_ids=list(range(N_CORES))).results

    outv = np.zeros((x.shape[0], S, D), np.float32)
    for c in range(N_CORES):
        b = c // HPG
        outv[b] += res[c]["out"]
    outv += np.asarray(bo, np.float32)
    return outv


# revision 106
# speedup vs baseline: 1.0246x; 1.0067x over previous
"""GQA kernel for Trainium2, 8 NeuronCores.

Problem: x[2,2048,2048] -> GQA(16 heads, 4 kv groups, dk=128) -> out[2,2048,2048]

Sharding: core c handles (batch b = c//4, kv-group g = c%4): the 4 query heads
of one group on one batch. Host sums the 4 per-group partial outputs per batch
(row-parallel O-proj reduction) and adds bo.

Numerics / engine split:
  - Q/K/V projections run as error-compensated fp8e4m3 DoubleRow matmuls: the
    host splits x (*16) and W (*64) into hi+lo fp8 pairs (scaled to dodge
    fp8's subnormal floor); pass 1 contracts hi*hi over chunk pairs, pass 2
    puts (w_hi*x_lo + w_lo*x_hi) in the two DoubleRow slots of one
    instruction.  0.75 cycles per 128-deep contraction vs 1.0 for bf16, at
    ~bf16 accuracy.
  - O-projection uses the same compensated-fp8 DoubleRow trick (aot split
    into hi/lo on DVE, Wo split on host).
  - scores stay bf16.  attn*V: heads 0-1 of each group use fp8 attn probs
    (exp emitted as fp8e4m3, scaled by 1/2 to fit the 240 max) against
    hi/lo-compensated fp8 V via DoubleRow (2x); heads 2-3 stay bf16.  This
    splits the fp8-attn quantization error in half: measured 1.44e-2 L2
    vs the 2e-2 budget.
  - ACT engine does the softmax Exp (PSUM f32 -> SBUF bf16/fp8).
  - DVE does bias-add copies, softmax normalize, transpose copies, output
    scaling; the final-block drain alternates output copies onto ACT.
  - Emission is software-pipelined: each attention unit (q-block, head)
    interleaves its 16 score matmuls with AV/Qproj/Oproj filler work so the
    PE never stalls waiting for ACT; the projection pre-phase is paced to
    the serialized DMA arrival order of x chunks and weight slices.
"""

import math
from collections import deque

import numpy as np
import ml_dtypes

import concourse.bass as bass
import concourse.mybir as mybir
import concourse.tile as tile
from concourse import bacc
from concourse.bass_utils import run_bass_kernel_spmd
from concourse.masks import make_identity

F32 = mybir.dt.float32
BF16 = mybir.dt.bfloat16
F8 = mybir.dt.float8e4

D = 2048          # d_model
S = 2048          # seq len
DK = 128          # head dim
HPG = 4           # heads per kv group
QCOLS = HPG * DK  # 512 q columns per core
N_CORES = 8
SCALE = 1.0 / math.sqrt(DK)

ND = D // 128     # 16 contraction chunks for projections
NSK = S // 128    # 16 key chunks
SQB = 512         # q-block (scores psum free size)
NJ = S // SQB     # 4 q-blocks
NSUB = SQB // 128  # 4 128-row subtiles per q-block
DR = mybir.MatmulPerfMode.DoubleRow
X_SCALE = 16.0   # fp8 hi/lo split scales (avoid subnormal underflow of lo)
W_SCALE = 64.0
INV_PROJ_SCALE = 1.0 / (X_SCALE * W_SCALE)
AO_SCALE = 16.0  # attn-out fp8 scale (via 1/16 ones column)
WO_SCALE = 64.0  # Wo fp8 hi/lo scale
INV_O_SCALE = 1.0 / (AO_SCALE * WO_SCALE)
FP8_HEADS = (0, 1, 2)    # heads whose attn probs are fp8 (DoubleRow AV)
ATTN8_BIAS = math.log(0.5)  # exp scaled by 0.5 so max ~147 fits fp8e4m3


def build_program(debug_dumps=False):
    nc = bacc.Bacc("TRN2", target_bir_lowering=False, debug=False,
                   num_devices=N_CORES)

    # fp8 hi/lo pair layouts; slot order: x=(lo,hi), w=(hi,lo) so that
    #   pass1: lhsT=w[:,2t:2t+2,0,:]  rhs=x[:,2t:2t+2,1,:]  -> hi*hi pairs
    #   pass2: lhsT=w[:,c,:,:]        rhs=x[:,c,:,:]        -> hi*lo + lo*hi
    xmix = nc.dram_tensor("xmix", [128, ND, 2, S], F8, kind="ExternalInput").ap()
    wqmix = nc.dram_tensor("wqmix", [128, ND, 2, QCOLS], F8, kind="ExternalInput").ap()
    # K/V weights slot-major: innermost run is ND*DK = 2KB per (partition,
    # slot), so the startup-critical transfers avoid the small-descriptor
    # latency penalty (1.45us each instead of 2.9us).
    wkmix = nc.dram_tensor("wkmix", [128, 2, ND, DK], F8, kind="ExternalInput").ap()
    wvmix = nc.dram_tensor("wvmix", [128, 2, ND, DK], F8, kind="ExternalInput").ap()
    wo = nc.dram_tensor("wo", [128, HPG, 2, D], F8, kind="ExternalInput").ap()
    bq = nc.dram_tensor("bq", [128, HPG], F32, kind="ExternalInput").ap()
    bk = nc.dram_tensor("bk", [128, 1], F32, kind="ExternalInput").ap()
    bv = nc.dram_tensor("bv", [128, 1], F32, kind="ExternalInput").ap()
    out = nc.dram_tensor("out", [S, D], F32, kind="ExternalOutput").ap()
    if debug_dumps:
        dbg_kt = nc.dram_tensor("dbg_kt", [128, S], BF16, kind="ExternalOutput").ap()
        dbg_qt = nc.dram_tensor("dbg_qt", [128, HPG, S], BF16, kind="ExternalOutput").ap()
        dbg_vones = nc.dram_tensor("dbg_vones", [128, NSK, 132], BF16, kind="ExternalOutput").ap()
        dbg_attn = nc.dram_tensor("dbg_attn", [128, SQB], BF16, kind="ExternalOutput").ap()
        dbg_aot = nc.dram_tensor("dbg_aot", [128, HPG, 2, SQB], F8, kind="ExternalOutput").ap()

    with tile.TileContext(nc) as tc:
        with (
            tc.tile_pool(name="singles", bufs=1) as singles,
            tc.tile_pool(name="attn", bufs=26) as attnpool,
            tc.tile_pool(name="aot", bufs=2) as aotpool,
            tc.tile_pool(name="ao", bufs=4) as aopool,
            tc.tile_pool(name="osb", bufs=4) as outpool,
            tc.tile_pool(name="small", bufs=6) as smallpool,
            tc.tile_pool(name="psSC", bufs=2, space="PSUM") as psSC,
            tc.tile_pool(name="psPO", bufs=2, space="PSUM") as psPO,
            tc.tile_pool(name="psAV", bufs=2, space="PSUM") as psAV,
        ):
            # ---- resident inputs, ordered to pace the projection pre-phase
            # (DMA engines are serial: deliver exactly what the PE needs
            # next) ----
            wk_sb = singles.tile([128, 2, ND, DK], F8)
            nc.sync.dma_start(out=wk_sb, in_=wkmix)
            bk_sb = singles.tile([128, 1], F32)
            nc.sync.dma_start(out=bk_sb, in_=bk)
            x_sb = singles.tile([128, ND, 2, S], F8)
            nc.sync.dma_start(out=x_sb[:, 0:4, :, 0:SQB],
                              in_=xmix[:, 0:4, :, 0:SQB])
            nc.sync.dma_start(out=x_sb[:, 4:8, :, 0:SQB],
                              in_=xmix[:, 4:8, :, 0:SQB])
            wv_sb = singles.tile([128, 2, ND, DK], F8)
            nc.sync.dma_start(out=wv_sb, in_=wvmix)
            bv_sb = singles.tile([128, 1], F32)
            nc.sync.dma_start(out=bv_sb, in_=bv)
            nc.sync.dma_start(out=x_sb[:, 8:12, :, 0:SQB],
                              in_=xmix[:, 8:12, :, 0:SQB])
            nc.sync.dma_start(out=x_sb[:, 12:16, :, 0:SQB],
                              in_=xmix[:, 12:16, :, 0:SQB])
            nc.sync.dma_start(out=x_sb[:, 0:8, :, SQB:2 * SQB],
                              in_=xmix[:, 0:8, :, SQB:2 * SQB])
            nc.sync.dma_start(out=x_sb[:, 8:16, :, SQB:2 * SQB],
                              in_=xmix[:, 8:16, :, SQB:2 * SQB])
            wq_sb = singles.tile([128, ND, 2, QCOLS], F8)
            bq_sb = singles.tile([128, HPG], F32)
            nc.sync.dma_start(out=wq_sb[:, :, :, 0:128], in_=wqmix[:, :, :, 0:128])
            nc.sync.dma_start(out=bq_sb, in_=bq)
            nc.sync.dma_start(out=x_sb[:, 0:8, :, 2 * SQB:3 * SQB],
                              in_=xmix[:, 0:8, :, 2 * SQB:3 * SQB])
            nc.sync.dma_start(out=x_sb[:, 8:16, :, 2 * SQB:3 * SQB],
                              in_=xmix[:, 8:16, :, 2 * SQB:3 * SQB])
            nc.sync.dma_start(out=wq_sb[:, :, :, 128:256], in_=wqmix[:, :, :, 128:256])
            nc.sync.dma_start(out=x_sb[:, 0:8, :, 3 * SQB:4 * SQB],
                              in_=xmix[:, 0:8, :, 3 * SQB:4 * SQB])
            nc.sync.dma_start(out=x_sb[:, 8:16, :, 3 * SQB:4 * SQB],
                              in_=xmix[:, 8:16, :, 3 * SQB:4 * SQB])
            nc.sync.dma_start(out=wq_sb[:, :, :, 256:384], in_=wqmix[:, :, :, 256:384])
            nc.sync.dma_start(out=wq_sb[:, :, :, 384:512], in_=wqmix[:, :, :, 384:512])
            wo_sb = singles.tile([128, HPG, 2, D], F8)
            nc.sync.dma_start(out=wo_sb, in_=wo)

            ident16 = singles.tile([128, 128], BF16)
            make_identity(nc, ident16)
            ln_half = singles.tile([128, 1], F32)
            nc.vector.memset(ln_half, ATTN8_BIAS)

            qt_sb = singles.tile([128, HPG, S], BF16)    # QT per head [dk, S]
            kt_sb = singles.tile([128, S], BF16)         # KT [dk, S]
            vt_sb = singles.tile([128, S], BF16)         # VT [dk, S]
            vones = singles.tile([128, NSK, 132], BF16)  # [V | 1/16] per key chunk
            nc.vector.memset(vones[:, :, 128:129], 1.0 / AO_SCALE)
            # fp8 hi/lo copies of [V | 1/16] for the DoubleRow AV path
            vones8h = singles.tile([128, NSK, 132], F8)
            nc.vector.memset(vones8h[:, :, 128:129], 1.0 / AO_SCALE)
            vones8l = singles.tile([128, NSK, 132], F8)
            nc.vector.memset(vones8l[:, :, 128:129], 0.0)

            # ---- helper emitters ----
            def comp_proj(w_ap, sl, psum, slot_major=False):
                """Accumulate compensated-fp8 projection of x block sl into
                psum: per chunk pair, hi*hi over the pair then the two cross
                terms (chunk demand is monotonic, so the first pair can start
                as soon as the first half of an x chunk-block lands)."""
                n = ND // 2
                for t in range(n):
                    lhsT = (w_ap[:, 0, 2 * t:2 * t + 2, :] if slot_major
                            else w_ap[:, 2 * t:2 * t + 2, 0, :])
                    nc.tensor.matmul(
                        psum, lhsT=lhsT,
                        rhs=x_sb[:, 2 * t:2 * t + 2, 1, sl],
                        start=(t == 0), stop=False, perf_mode=DR)
                    for c in (2 * t, 2 * t + 1):
                        lhsT = (w_ap[:, :, c, :] if slot_major
                                else w_ap[:, c, :, :])
                        nc.tensor.matmul(
                            psum, lhsT=lhsT,
                            rhs=x_sb[:, c, :, sl],
                            start=False, stop=(c == ND - 1), perf_mode=DR)

            def kv_proj_block(jb, which):
                sl = bass.ts(jb, SQB)
                p = psPO.tile([128, SQB], F32, tag="po", name=f"pj{jb}{which}")
                if which == "k":
                    comp_proj(wk_sb, sl, p, slot_major=True)
                    nc.vector.tensor_scalar(kt_sb[:, sl], p, INV_PROJ_SCALE,
                                            bk_sb, op0=mybir.AluOpType.mult,
                                            op1=mybir.AluOpType.add)
                else:
                    comp_proj(wv_sb, sl, p, slot_major=True)
                    nc.vector.tensor_scalar(vt_sb[:, sl], p, INV_PROJ_SCALE,
                                            bv_sb, op0=mybir.AluOpType.mult,
                                            op1=mybir.AluOpType.add)

            def q_proj_head(j, h):
                sl = bass.ts(j, SQB)
                p = psPO.tile([128, SQB], F32, tag="po", name=f"q{j}{h}")
                w = wq_sb[:, :, :, bass.ts(h, 128)]
                comp_proj(w, sl, p)
                nc.vector.tensor_scalar(qt_sb[:, h, sl], p, INV_PROJ_SCALE,
                                        bq_sb[:, h:h + 1],
                                        op0=mybir.AluOpType.mult,
                                        op1=mybir.AluOpType.add)

            def v_transpose(sk):
                pt = psAV.tile([128, 128], BF16, tag="av")
                nc.tensor.transpose(pt, vt_sb[:, bass.ts(sk, 128)], ident16)
                nc.vector.tensor_copy(vones[:, sk, 0:128], pt)
                hi = vones8h[:, sk, 0:128]
                nc.vector.tensor_copy(hi, pt)
                nc.vector.tensor_tensor(vones8l[:, sk, 0:128], pt, hi,
                                        mybir.AluOpType.subtract)

            attn_tiles = {}
            aot_tiles = {}

            def av_sub(j, h, sub):
                """attn@[V|1/16] for 128 q rows, then normalize (scaled by 16)
                + transpose + fp8 hi/lo split into aot_tiles[j]."""
                mode8, tiles = attn_tiles[(j, h)]
                pav = psAV.tile([128, 132], F32, tag="av")
                if mode8:
                    ssl = bass.ts(sub, 128)
                    for t in range(NSK // 2):
                        nc.tensor.matmul(
                            pav[:, 0:129], lhsT=tiles[t][:, :, ssl],
                            rhs=vones8h[:, 2 * t:2 * t + 2, 0:129],
                            start=(t == 0), stop=False, perf_mode=DR)
                    for t in range(NSK // 2):
                        nc.tensor.matmul(
                            pav[:, 0:129], lhsT=tiles[t][:, :, ssl],
                            rhs=vones8l[:, 2 * t:2 * t + 2, 0:129],
                            start=False, stop=(t == NSK // 2 - 1),
                            perf_mode=DR)
                else:
                    for sk in range(NSK):
                        nc.tensor.matmul(
                            pav[:, 0:129],
                            lhsT=tiles[sk // 2][:, sk % 2, bass.ts(sub, 128)],
                            rhs=vones[:, sk, 0:129],
                            start=(sk == 0), stop=(sk == NSK - 1))
                recip = smallpool.tile([128, 1], F32)
                nc.vector.reciprocal(recip, pav[:, 128:129])
                ao = aopool.tile([128, 128], BF16)
                nc.vector.tensor_scalar_mul(ao, pav[:, 0:128], recip)
                pt = psAV.tile([128, 128], BF16, tag="av")
                nc.tensor.transpose(pt, ao, ident16)
                aot = aot_tiles[j]
                hi = aot[:, h, 1, bass.ts(sub, 128)]
                nc.vector.tensor_copy(hi, pt)
                nc.vector.tensor_tensor(
                    aot[:, h, 0, bass.ts(sub, 128)], pt, hi,
                    mybir.AluOpType.subtract)

            def oproj_sub(j, sub, dcs, copy_engine="dve", width=512):
                """Compensated-fp8 O-projection for q rows [j*SQB + sub*128,
                +128), d_model chunks dcs (each `width` wide)."""
                aot = aot_tiles[j]
                for dc in dcs:
                    po = psPO.tile([128, width], F32, tag="po", name="po")
                    dsl = bass.ts(dc, width)
                    ssl = bass.ts(sub, 128)
                    for t in range(HPG // 2):
                        nc.tensor.matmul(
                            po, lhsT=aot[:, 2 * t:2 * t + 2, 1, ssl],
                            rhs=wo_sb[:, 2 * t:2 * t + 2, 0, dsl],
                            start=(t == 0), stop=False, perf_mode=DR)
                    for h in range(HPG):
                        nc.tensor.matmul(
                            po, lhsT=aot[:, h, :, ssl],
                            rhs=wo_sb[:, h, :, dsl],
                            start=False, stop=(h == HPG - 1), perf_mode=DR)
                    osb = outpool.tile([128, width], F32, tag="osb", name="osb")
                    eng = copy_engine
                    if eng == "mix":
                        eng = "act" if dc % 2 == 0 else "dve"
                    if eng == "act":
                        nc.scalar.activation(
                            out=osb, in_=po,
                            func=mybir.ActivationFunctionType.Copy,
                            scale=INV_O_SCALE)
                    else:
                        nc.vector.tensor_scalar_mul(osb, po, INV_O_SCALE)
                    nc.sync.dma_start(
                        out=out[j * SQB + sub * 128: j * SQB + (sub + 1) * 128,
                                dsl],
                        in_=osb)

            def emit_scores_pair(j, h, p, tiles, mode8, u):
                """Scores for sk chunks 2p, 2p+1 into one 2-bank PSUM tile,
                exp'd by a single 1024-free ACT instruction into a pair
                tile (fp8 or bf16)."""
                ps = psSC.tile([128, 2, SQB], F32, tag="sc", name=f"sc{u}_{p}")
                for i in (0, 1):
                    nc.tensor.matmul(ps[:, i, :],
                                     lhsT=kt_sb[:, bass.ts(2 * p + i, 128)],
                                     rhs=qt_sb[:, h, bass.ts(j, SQB)],
                                     start=True, stop=True)
                if mode8:
                    a8 = attnpool.tile([128, 2, SQB], F8, tag="a8",
                                       bufs=18, name=f"a8_{u}_{p}")
                    nc.scalar.activation(
                        out=a8, in_=ps,
                        func=mybir.ActivationFunctionType.Exp,
                        scale=SCALE, bias=ln_half)
                    tiles.append(a8)
                else:
                    a = attnpool.tile([128, 2, SQB], BF16, tag="a", bufs=12,
                                      name=f"a_{u}_{p}")
                    nc.scalar.activation(
                        out=a, in_=ps,
                        func=mybir.ActivationFunctionType.Exp, scale=SCALE)
                    tiles.append(a)

            # ---- pre-phase: all projections for block 0 + K/V, paced to the
            # DMA arrival order of the x chunks and wq head slices; scores of
            # unit (0,0) are woven in as the K blocks they need complete, so
            # the ACT engine starts its softmax work ~20us earlier.
            # Alternate psum pools so the next block's matmuls don't wait on
            # the previous block's PSUM->SBUF copy (psPO is idle until the
            # first O-projection, well after this phase).
            pre = [("kv", 0, "k"), ("kv", 0, "v"), ("vt", 0, None),
                   ("kv", 1, "k"), ("kv", 1, "v"), ("vt", 1, None),
                   ("q", 0, 0),
                   ("kv", 2, "k"), ("kv", 2, "v"), ("vt", 2, None),
                   ("q", 0, 1),
                   ("kv", 3, "k"), ("kv", 3, "v"), ("vt", 3, None),
                   ("q", 0, 2), ("q", 0, 3)]
            for kind, a, b in pre:
                if kind == "kv":
                    kv_proj_block(a, b)
                elif kind == "q":
                    q_proj_head(a, b)
                else:
                    for sk in range(4 * a, 4 * a + 4):
                        v_transpose(sk)

            # ---- attention units, software pipelined ----
            units = [(j, h) for j in range(NJ) for h in range(HPG)]

            def unit_fillers(u):
                """PE filler closures for unit u (consumed between score
                matmuls)."""
                j, h = units[u]
                fill = []
                if u >= 1:
                    pj, ph = units[u - 1]
                    avs = [lambda pj=pj, ph=ph, sub=sub: av_sub(pj, ph, sub)
                           for sub in range(NSUB)]
                    ops = []
                    if j >= 1:
                        # O-projection for block j-1, sub h (split in two);
                        # independent of this unit's ACT work, so lead with it
                        # when aot(j-1) is already complete (h > 0).
                        ops = [lambda j=j, h=h: oproj_sub(j - 1, h, range(0, 2)),
                               lambda j=j, h=h: oproj_sub(j - 1, h, range(2, 4))]
                    if h == 0:
                        fill = avs + ops
                    else:
                        fill = [ops[0], avs[0], avs[1], ops[1],
                                avs[2], avs[3]] if ops else avs
                    if u >= 2:
                        del attn_tiles[units[u - 2]]
                # Q projection four units ahead (block 0 is in the pre-phase).
                if u + 4 < len(units):
                    nj, nh = units[u + 4]
                    fill.append(lambda nj=nj, nh=nh: q_proj_head(nj, nh))
                return deque(fill)

            for u, (j, h) in enumerate(units):
                if h == 0:
                    aot_j = aotpool.tile([128, HPG, 2, SQB], F8, tag="aot",
                                         name=f"aot{j}")
                    aot_tiles[j] = aot_j
                fill = unit_fillers(u)
                mode8 = h in FP8_HEADS
                tiles = []
                for p in range(NSK // 2):
                    emit_scores_pair(j, h, p, tiles, mode8, u)
                    if fill:
                        fill.popleft()()
                while fill:
                    fill.popleft()()
                attn_tiles[(j, h)] = (mode8, tiles)

            # ---- drain: last unit's AV + O-projection of last block,
            # software-pipelined (AV one sub ahead of its O-projection) with
            # output copies on the otherwise-idle ACT engine ----
            av_sub(NJ - 1, HPG - 1, 0)
            for sub in range(NSUB):
                if sub + 1 < NSUB:
                    av_sub(NJ - 1, HPG - 1, sub + 1)
                oproj_sub(NJ - 1, sub, range(0, 4), copy_engine="mix")

            if debug_dumps:
                nc.sync.dma_start(out=dbg_kt, in_=kt_sb)
                nc.sync.dma_start(out=dbg_qt, in_=qt_sb)
                nc.sync.dma_start(out=dbg_vones, in_=vones)
                nc.sync.dma_start(out=dbg_attn, in_=attn_tiles[(NJ - 1, HPG - 1)][1][0])
                nc.sync.dma_start(out=dbg_aot, in_=aot_tiles[NJ - 1])

    nc.compile()
    return nc


_NC_CACHE = None


def _get_program():
    global _NC_CACHE
    if _NC_CACHE is None:
        _NC_CACHE = build_program()
    return _NC_CACHE


def _hi_lo(a):
    """Split float32 array into fp8e4m3 hi + lo parts."""
    hi = np.asarray(a, dtype=ml_dtypes.float8_e4m3)
    lo = np.asarray(a - hi.astype(np.float32), dtype=ml_dtypes.float8_e4m3)
    return hi, lo


def _mix(a, ncols, slot_order, scale):
    """[D, ncols] f32 -> [128, ND, 2, ncols] fp8 with given hi/lo slot order."""
    a3 = np.ascontiguousarray(a.reshape(ND, 128, ncols)) * np.float32(scale)
    hi, lo = _hi_lo(a3)
    parts = {"hi": hi, "lo": lo}
    stacked = np.stack([parts[slot_order[0]], parts[slot_order[1]]], axis=2)
    return np.ascontiguousarray(stacked.transpose(1, 0, 2, 3))


def _mix_kv(wk_g, wv_g):
    """Two [D, DK] f32 -> [128, ND, 2, 2, DK] fp8 (slot order hi,lo; then
    k/v)."""
    k = _mix(wk_g, DK, ("hi", "lo"), W_SCALE)
    v = _mix(wv_g, DK, ("hi", "lo"), W_SCALE)
    return np.ascontiguousarray(np.stack([k, v], axis=3))


def kernel(x, Wq, bq, Wk, bk, Wv, bv, Wo, bo):
    x = np.asarray(x, np.float32)
    Wq = np.asarray(Wq, np.float32)
    Wk = np.asarray(Wk, np.float32)
    Wv = np.asarray(Wv, np.float32)
    Wo = np.asarray(Wo, np.float32)
    nc = _get_program()

    xmix = [_mix(np.ascontiguousarray(x[b].T), S, ("lo", "hi"), X_SCALE)
            for b in range(x.shape[0])]

    in_maps = []
    for c in range(N_CORES):
        b, g = divmod(c, HPG)
        wo_g = Wo[g * QCOLS:(g + 1) * QCOLS, :].reshape(HPG, 128, D) * np.float32(WO_SCALE)
        wo_hi, wo_lo = _hi_lo(wo_g)
        wo_mix = np.stack([wo_hi, wo_lo], axis=2)  # [HPG, 128, 2, D]
        in_maps.append({
            "xmix": xmix[b],
            "wqmix": _mix(Wq[:, g * QCOLS:(g + 1) * QCOLS], QCOLS, ("hi", "lo"), W_SCALE),
            "wkmix": np.ascontiguousarray(
                _mix(Wk[:, g * DK:(g + 1) * DK], DK, ("hi", "lo"), W_SCALE)
                .transpose(0, 2, 1, 3)),
            "wvmix": np.ascontiguousarray(
                _mix(Wv[:, g * DK:(g + 1) * DK], DK, ("hi", "lo"), W_SCALE)
                .transpose(0, 2, 1, 3)),
            "wo": np.ascontiguousarray(wo_mix.transpose(1, 0, 2, 3)),
            "bq": np.ascontiguousarray(
                np.asarray(bq, np.float32)[g * QCOLS:(g + 1) * QCOLS]
                .reshape(HPG, 128).T),
            "bk": np.ascontiguousarray(
                np.asarray(bk, np.float32)[g * DK:(g + 1) * DK].reshape(128, 1)),
            "bv": np.ascontiguousarray(
                np.asarray(bv, np.float32)[g * DK:(g + 1) * DK].reshape(128, 1)),
        })

    res = run_bass_kernel_spmd(nc, in_maps, core_ids=list(range(N_CORES))).results

    outv = np.zeros((x.shape[0], S, D), np.float32)
    for c in range(N_CORES):
        b = c // HPG
        outv[b] += res[c]["out"]
    outv += np.asarray(bo, np.float32)
    return outv


# revision 107
# speedup vs baseline: 1.0339x; 1.0090x over previous
"""GQA kernel for Trainium2, 8 NeuronCores.

Problem: x[2,2048,2048] -> GQA(16 heads, 4 kv groups, dk=128) -> out[2,2048,2048]

Sharding: core c handles (batch b = c//4, kv-group g = c%4): the 4 query heads
of one group on one batch. Host sums the 4 per-group partial outputs per batch
(row-parallel O-proj reduction) and adds bo.

Numerics / engine split:
  - Q/K/V projections run as error-compensated fp8e4m3 DoubleRow matmuls: the
    host splits x (*16) and W (*64) into hi+lo fp8 pairs (scaled to dodge
    fp8's subnormal floor); pass 1 contracts hi*hi over chunk pairs, pass 2
    puts (w_hi*x_lo + w_lo*x_hi) in the two DoubleRow slots of one
    instruction.  0.75 cycles per 128-deep contraction vs 1.0 for bf16, at
    ~bf16 accuracy.
  - O-projection uses the same compensated-fp8 DoubleRow trick (aot split
    into hi/lo on DVE, Wo split on host).
  - scores stay bf16.  attn*V: heads 0-1 of each group use fp8 attn probs
    (exp emitted as fp8e4m3, scaled by 1/2 to fit the 240 max) against
    hi/lo-compensated fp8 V via DoubleRow (2x); heads 2-3 stay bf16.  This
    splits the fp8-attn quantization error in half: measured 1.44e-2 L2
    vs the 2e-2 budget.
  - ACT engine does the softmax Exp (PSUM f32 -> SBUF bf16/fp8).
  - DVE does bias-add copies, softmax normalize, transpose copies, output
    scaling; the final-block drain alternates output copies onto ACT.
  - Emission is software-pipelined: each attention unit (q-block, head)
    interleaves its 16 score matmuls with AV/Qproj/Oproj filler work so the
    PE never stalls waiting for ACT; the projection pre-phase is paced to
    the serialized DMA arrival order of x chunks and weight slices.
"""

import math
from collections import deque

import numpy as np
import ml_dtypes

import concourse.bass as bass
import concourse.mybir as mybir
import concourse.tile as tile
from concourse import bacc
from concourse.bass_utils import run_bass_kernel_spmd
from concourse.masks import make_identity

F32 = mybir.dt.float32
BF16 = mybir.dt.bfloat16
F8 = mybir.dt.float8e4

D = 2048          # d_model
S = 2048          # seq len
DK = 128          # head dim
HPG = 4           # heads per kv group
QCOLS = HPG * DK  # 512 q columns per core
N_CORES = 8
SCALE = 1.0 / math.sqrt(DK)

ND = D // 128     # 16 contraction chunks for projections
NSK = S // 128    # 16 key chunks
SQB = 512         # q-block (scores psum free size)
NJ = S // SQB     # 4 q-blocks
NSUB = SQB // 128  # 4 128-row subtiles per q-block
DR = mybir.MatmulPerfMode.DoubleRow
X_SCALE = 16.0   # fp8 hi/lo split scales (avoid subnormal underflow of lo)
W_SCALE = 64.0
INV_PROJ_SCALE = 1.0 / (X_SCALE * W_SCALE)
AO_SCALE = 16.0  # attn-out fp8 scale (via 1/16 ones column)
WO_SCALE = 64.0  # Wo fp8 hi/lo scale
INV_O_SCALE = 1.0 / (AO_SCALE * WO_SCALE)
FP8_HEADS = (0, 1, 2)    # heads whose attn probs are fp8 (DoubleRow AV)
ATTN8_BIAS = math.log(0.5)  # exp scaled by 0.5 so max ~147 fits fp8e4m3


def build_program(debug_dumps=False):
    nc = bacc.Bacc("TRN2", target_bir_lowering=False, debug=False,
                   num_devices=N_CORES)

    # fp8 hi/lo pair layouts; slot order: x=(lo,hi), w=(hi,lo) so that
    #   pass1: lhsT=w[:,2t:2t+2,0,:]  rhs=x[:,2t:2t+2,1,:]  -> hi*hi pairs
    #   pass2: lhsT=w[:,c,:,:]        rhs=x[:,c,:,:]        -> hi*lo + lo*hi
    xmix = nc.dram_tensor("xmix", [128, ND, 2, S], F8, kind="ExternalInput").ap()
    wqmix = nc.dram_tensor("wqmix", [128, ND, 2, QCOLS], F8, kind="ExternalInput").ap()
    # K/V weights slot-major: innermost run is ND*DK = 2KB per (partition,
    # slot), so the startup-critical transfers avoid the small-descriptor
    # latency penalty (1.45us each instead of 2.9us).
    wkmix = nc.dram_tensor("wkmix", [128, 2, ND, DK], F8, kind="ExternalInput").ap()
    wvmix = nc.dram_tensor("wvmix", [128, 2, ND, DK], F8, kind="ExternalInput").ap()
    wo = nc.dram_tensor("wo", [128, HPG, 2, D], F8, kind="ExternalInput").ap()
    bq = nc.dram_tensor("bq", [128, HPG], F32, kind="ExternalInput").ap()
    bk = nc.dram_tensor("bk", [128, 1], F32, kind="ExternalInput").ap()
    bv = nc.dram_tensor("bv", [128, 1], F32, kind="ExternalInput").ap()
    out = nc.dram_tensor("out", [S, D], F32, kind="ExternalOutput").ap()
    if debug_dumps:
        dbg_kt = nc.dram_tensor("dbg_kt", [128, S], BF16, kind="ExternalOutput").ap()
        dbg_qt = nc.dram_tensor("dbg_qt", [128, HPG, S], BF16, kind="ExternalOutput").ap()
        dbg_vones = nc.dram_tensor("dbg_vones", [128, NSK, 132], BF16, kind="ExternalOutput").ap()
        dbg_attn = nc.dram_tensor("dbg_attn", [128, SQB], BF16, kind="ExternalOutput").ap()
        dbg_aot = nc.dram_tensor("dbg_aot", [128, HPG, 2, SQB], F8, kind="ExternalOutput").ap()

    with tile.TileContext(nc) as tc:
        with (
            tc.tile_pool(name="singles", bufs=1) as singles,
            tc.tile_pool(name="attn", bufs=26) as attnpool,
            tc.tile_pool(name="aot", bufs=2) as aotpool,
            tc.tile_pool(name="ao", bufs=4) as aopool,
            tc.tile_pool(name="osb", bufs=4) as outpool,
            tc.tile_pool(name="small", bufs=6) as smallpool,
            tc.tile_pool(name="psSC", bufs=2, space="PSUM") as psSC,
            tc.tile_pool(name="psPO", bufs=2, space="PSUM") as psPO,
            tc.tile_pool(name="psAV", bufs=2, space="PSUM") as psAV,
        ):
            # ---- resident inputs, ordered to pace the projection pre-phase
            # (DMA engines are serial: deliver exactly what the PE needs
            # next) ----
            wk_sb = singles.tile([128, 2, ND, DK], F8)
            nc.sync.dma_start(out=wk_sb, in_=wkmix)
            bk_sb = singles.tile([128, 1], F32)
            nc.sync.dma_start(out=bk_sb, in_=bk)
            x_sb = singles.tile([128, ND, 2, S], F8)
            nc.sync.dma_start(out=x_sb[:, 0:4, :, 0:SQB],
                              in_=xmix[:, 0:4, :, 0:SQB])
            nc.sync.dma_start(out=x_sb[:, 4:8, :, 0:SQB],
                              in_=xmix[:, 4:8, :, 0:SQB])
            wv_sb = singles.tile([128, 2, ND, DK], F8)
            nc.sync.dma_start(out=wv_sb, in_=wvmix)
            bv_sb = singles.tile([128, 1], F32)
            nc.sync.dma_start(out=bv_sb, in_=bv)
            nc.sync.dma_start(out=x_sb[:, 8:12, :, 0:SQB],
                              in_=xmix[:, 8:12, :, 0:SQB])
            nc.sync.dma_start(out=x_sb[:, 12:16, :, 0:SQB],
                              in_=xmix[:, 12:16, :, 0:SQB])
            nc.sync.dma_start(out=x_sb[:, 0:8, :, SQB:2 * SQB],
                              in_=xmix[:, 0:8, :, SQB:2 * SQB])
            nc.sync.dma_start(out=x_sb[:, 8:16, :, SQB:2 * SQB],
                              in_=xmix[:, 8:16, :, SQB:2 * SQB])
            wq_sb = singles.tile([128, ND, 2, QCOLS], F8)
            bq_sb = singles.tile([128, HPG], F32)
            nc.sync.dma_start(out=wq_sb[:, :, :, 0:128], in_=wqmix[:, :, :, 0:128])
            nc.sync.dma_start(out=bq_sb, in_=bq)
            nc.sync.dma_start(out=x_sb[:, 0:8, :, 2 * SQB:3 * SQB],
                              in_=xmix[:, 0:8, :, 2 * SQB:3 * SQB])
            nc.sync.dma_start(out=x_sb[:, 8:16, :, 2 * SQB:3 * SQB],
                              in_=xmix[:, 8:16, :, 2 * SQB:3 * SQB])
            nc.sync.dma_start(out=wq_sb[:, :, :, 128:256], in_=wqmix[:, :, :, 128:256])
            nc.sync.dma_start(out=x_sb[:, 0:8, :, 3 * SQB:4 * SQB],
                              in_=xmix[:, 0:8, :, 3 * SQB:4 * SQB])
            nc.sync.dma_start(out=x_sb[:, 8:16, :, 3 * SQB:4 * SQB],
                              in_=xmix[:, 8:16, :, 3 * SQB:4 * SQB])
            nc.sync.dma_start(out=wq_sb[:, :, :, 256:384], in_=wqmix[:, :, :, 256:384])
            nc.sync.dma_start(out=wq_sb[:, :, :, 384:512], in_=wqmix[:, :, :, 384:512])
            wo_sb = singles.tile([128, HPG, 2, D], F8)
            nc.sync.dma_start(out=wo_sb, in_=wo)

            ident16 = singles.tile([128, 128], BF16)
            make_identity(nc, ident16)
            ln_half = singles.tile([128, 1], F32)
            nc.vector.memset(ln_half, ATTN8_BIAS)

            qt_sb = singles.tile([128, HPG, S], BF16)    # QT per head [dk, S]
            kt_sb = singles.tile([128, S], BF16)         # KT [dk, S]
            vt_sb = singles.tile([128, S], BF16)         # VT [dk, S]
            vones = singles.tile([128, NSK, 132], BF16)  # [V | 1/16] per key chunk
            nc.vector.memset(vones[:, :, 128:129], 1.0 / AO_SCALE)
            # fp8 hi/lo copies of [V | 1/16] for the DoubleRow AV path
            vones8h = singles.tile([128, NSK, 132], F8)
            nc.vector.memset(vones8h[:, :, 128:129], 1.0 / AO_SCALE)
            vones8l = singles.tile([128, NSK, 132], F8)
            nc.vector.memset(vones8l[:, :, 128:129], 0.0)

            # ---- helper emitters ----
            def comp_proj(w_ap, sl, psum, slot_major=False):
                """Accumulate compensated-fp8 projection of x block sl into
                psum: per chunk pair, hi*hi over the pair then the two cross
                terms (chunk demand is monotonic, so the first pair can start
                as soon as the first half of an x chunk-block lands)."""
                n = ND // 2
                for t in range(n):
                    lhsT = (w_ap[:, 0, 2 * t:2 * t + 2, :] if slot_major
                            else w_ap[:, 2 * t:2 * t + 2, 0, :])
                    nc.tensor.matmul(
                        psum, lhsT=lhsT,
                        rhs=x_sb[:, 2 * t:2 * t + 2, 1, sl],
                        start=(t == 0), stop=False, perf_mode=DR)
                    for c in (2 * t, 2 * t + 1):
                        lhsT = (w_ap[:, :, c, :] if slot_major
                                else w_ap[:, c, :, :])
                        nc.tensor.matmul(
                            psum, lhsT=lhsT,
                            rhs=x_sb[:, c, :, sl],
                            start=False, stop=(c == ND - 1), perf_mode=DR)

            def kv_proj_block(jb, which):
                sl = bass.ts(jb, SQB)
                p = psPO.tile([128, SQB], F32, tag="po", name=f"pj{jb}{which}")
                if which == "k":
                    comp_proj(wk_sb, sl, p, slot_major=True)
                    nc.vector.tensor_scalar(kt_sb[:, sl], p, INV_PROJ_SCALE,
                                            bk_sb, op0=mybir.AluOpType.mult,
                                            op1=mybir.AluOpType.add)
                else:
                    comp_proj(wv_sb, sl, p, slot_major=True)
                    nc.vector.tensor_scalar(vt_sb[:, sl], p, INV_PROJ_SCALE,
                                            bv_sb, op0=mybir.AluOpType.mult,
                                            op1=mybir.AluOpType.add)

            def q_proj_head(j, h):
                sl = bass.ts(j, SQB)
                p = psPO.tile([128, SQB], F32, tag="po", name=f"q{j}{h}")
                w = wq_sb[:, :, :, bass.ts(h, 128)]
                comp_proj(w, sl, p)
                nc.vector.tensor_scalar(qt_sb[:, h, sl], p, INV_PROJ_SCALE,
                                        bq_sb[:, h:h + 1],
                                        op0=mybir.AluOpType.mult,
                                        op1=mybir.AluOpType.add)

            def v_transpose(sk):
                pt = psAV.tile([128, 128], BF16, tag="av")
                nc.tensor.transpose(pt, vt_sb[:, bass.ts(sk, 128)], ident16)
                nc.vector.tensor_copy(vones[:, sk, 0:128], pt)
                hi = vones8h[:, sk, 0:128]
                nc.vector.tensor_copy(hi, pt)
                nc.vector.tensor_tensor(vones8l[:, sk, 0:128], pt, hi,
                                        mybir.AluOpType.subtract)

            attn_tiles = {}
            aot_tiles = {}

            def av_sub(j, h, sub):
                """attn@[V|1/16] for 128 q rows, then normalize (scaled by 16)
                + transpose + fp8 hi/lo split into aot_tiles[j]."""
                mode8, tiles = attn_tiles[(j, h)]
                pav = psAV.tile([128, 132], F32, tag="av")
                if mode8:
                    ssl = bass.ts(sub, 128)
                    for t in range(NSK // 2):
                        nc.tensor.matmul(
                            pav[:, 0:129], lhsT=tiles[t][:, :, ssl],
                            rhs=vones8h[:, 2 * t:2 * t + 2, 0:129],
                            start=(t == 0), stop=False, perf_mode=DR)
                    for t in range(NSK // 2):
                        nc.tensor.matmul(
                            pav[:, 0:129], lhsT=tiles[t][:, :, ssl],
                            rhs=vones8l[:, 2 * t:2 * t + 2, 0:129],
                            start=False, stop=(t == NSK // 2 - 1),
                            perf_mode=DR)
                else:
                    for sk in range(NSK):
                        nc.tensor.matmul(
                            pav[:, 0:129],
                            lhsT=tiles[sk // 2][:, sk % 2, bass.ts(sub, 128)],
                            rhs=vones[:, sk, 0:129],
                            start=(sk == 0), stop=(sk == NSK - 1))
                recip = smallpool.tile([128, 1], F32)
                nc.vector.reciprocal(recip, pav[:, 128:129])
                ao = aopool.tile([128, 128], BF16)
                nc.vector.tensor_scalar_mul(ao, pav[:, 0:128], recip)
                pt = psAV.tile([128, 128], BF16, tag="av")
                nc.tensor.transpose(pt, ao, ident16)
                aot = aot_tiles[j]
                hi = aot[:, h, 1, bass.ts(sub, 128)]
                nc.vector.tensor_copy(hi, pt)
                nc.vector.tensor_tensor(
                    aot[:, h, 0, bass.ts(sub, 128)], pt, hi,
                    mybir.AluOpType.subtract)

            def oproj_sub(j, sub, dcs, copy_engine="dve", width=512):
                """Compensated-fp8 O-projection for q rows [j*SQB + sub*128,
                +128), d_model chunks dcs (each `width` wide)."""
                aot = aot_tiles[j]
                for dc in dcs:
                    po = psPO.tile([128, width], F32, tag="po", name="po")
                    dsl = bass.ts(dc, width)
                    ssl = bass.ts(sub, 128)
                    for t in range(HPG // 2):
                        nc.tensor.matmul(
                            po, lhsT=aot[:, 2 * t:2 * t + 2, 1, ssl],
                            rhs=wo_sb[:, 2 * t:2 * t + 2, 0, dsl],
                            start=(t == 0), stop=False, perf_mode=DR)
                    for h in range(HPG):
                        nc.tensor.matmul(
                            po, lhsT=aot[:, h, :, ssl],
                            rhs=wo_sb[:, h, :, dsl],
                            start=False, stop=(h == HPG - 1), perf_mode=DR)
                    osb = outpool.tile([128, width], F32, tag="osb", name="osb")
                    eng = copy_engine
                    if eng == "mix":
                        eng = "act" if dc % 2 == 0 else "dve"
                    if eng == "act":
                        nc.scalar.activation(
                            out=osb, in_=po,
                            func=mybir.ActivationFunctionType.Copy,
                            scale=INV_O_SCALE)
                    else:
                        nc.vector.tensor_scalar_mul(osb, po, INV_O_SCALE)
                    nc.sync.dma_start(
                        out=out[j * SQB + sub * 128: j * SQB + (sub + 1) * 128,
                                dsl],
                        in_=osb)

            def emit_scores_pair(j, h, p, tiles, mode8, u):
                """Scores for sk chunks 2p, 2p+1 into one 2-bank PSUM tile,
                exp'd by a single 1024-free ACT instruction into a pair
                tile (fp8 or bf16)."""
                ps = psSC.tile([128, 2, SQB], F32, tag="sc", name=f"sc{u}_{p}")
                for i in (0, 1):
                    nc.tensor.matmul(ps[:, i, :],
                                     lhsT=kt_sb[:, bass.ts(2 * p + i, 128)],
                                     rhs=qt_sb[:, h, bass.ts(j, SQB)],
                                     start=True, stop=True)
                if mode8:
                    a8 = attnpool.tile([128, 2, SQB], F8, tag="a8",
                                       bufs=18, name=f"a8_{u}_{p}")
                    nc.scalar.activation(
                        out=a8, in_=ps,
                        func=mybir.ActivationFunctionType.Exp,
                        scale=SCALE, bias=ln_half)
                    tiles.append(a8)
                else:
                    a = attnpool.tile([128, 2, SQB], BF16, tag="a", bufs=12,
                                      name=f"a_{u}_{p}")
                    nc.scalar.activation(
                        out=a, in_=ps,
                        func=mybir.ActivationFunctionType.Exp, scale=SCALE)
                    tiles.append(a)

            # ---- pre-phase: all projections for block 0 + K/V, paced to the
            # DMA arrival order of the x chunks and wq head slices; scores of
            # unit (0,0) are woven in as the K blocks they need complete, so
            # the ACT engine starts its softmax work ~20us earlier.
            # Alternate psum pools so the next block's matmuls don't wait on
            # the previous block's PSUM->SBUF copy (psPO is idle until the
            # first O-projection, well after this phase).
            pre = [("kv", 0, "k"), ("kv", 0, "v"), ("vt", 0, None),
                   ("kv", 1, "k"), ("kv", 1, "v"), ("vt", 1, None),
                   ("q", 0, 0),
                   ("kv", 2, "k"), ("kv", 2, "v"), ("vt", 2, None),
                   ("q", 0, 1),
                   ("kv", 3, "k"), ("kv", 3, "v"), ("vt", 3, None),
                   ("q", 0, 2), ("q", 0, 3)]
            for kind, a, b in pre:
                if kind == "kv":
                    kv_proj_block(a, b)
                elif kind == "q":
                    q_proj_head(a, b)
                else:
                    for sk in range(4 * a, 4 * a + 4):
                        v_transpose(sk)

            # ---- attention units, software pipelined ----
            units = [(j, h) for j in range(NJ) for h in range(HPG)]

            def unit_fillers(u):
                """PE filler closures for unit u (consumed between score
                matmuls)."""
                j, h = units[u]
                fill = []
                if u >= 1:
                    pj, ph = units[u - 1]
                    avs = [lambda pj=pj, ph=ph, sub=sub: av_sub(pj, ph, sub)
                           for sub in range(NSUB)]
                    ops = []
                    if j >= 1:
                        # O-projection for block j-1, sub h (split in two);
                        # independent of this unit's ACT work, so lead with it
                        # when aot(j-1) is already complete (h > 0).
                        ops = [lambda j=j, h=h: oproj_sub(j - 1, h, range(0, 2)),
                               lambda j=j, h=h: oproj_sub(j - 1, h, range(2, 4))]
                    if h == 0:
                        fill = avs + ops
                    else:
                        fill = [ops[0], avs[0], avs[1], ops[1],
                                avs[2], avs[3]] if ops else avs
                    if u >= 2:
                        del attn_tiles[units[u - 2]]
                # Q projection four units ahead (block 0 is in the
                # pre-phase), placed mid-unit so its PSUM-ring slot drains
                # before the next unit's O-projection needs one.
                if u + 4 < len(units):
                    nj, nh = units[u + 4]
                    qp = lambda nj=nj, nh=nh: q_proj_head(nj, nh)
                    fill.insert(min(2, len(fill)), qp)
                return deque(fill)

            for u, (j, h) in enumerate(units):
                if h == 0:
                    aot_j = aotpool.tile([128, HPG, 2, SQB], F8, tag="aot",
                                         name=f"aot{j}")
                    aot_tiles[j] = aot_j
                fill = unit_fillers(u)
                mode8 = h in FP8_HEADS
                tiles = []
                for p in range(NSK // 2):
                    emit_scores_pair(j, h, p, tiles, mode8, u)
                    if fill:
                        fill.popleft()()
                while fill:
                    fill.popleft()()
                attn_tiles[(j, h)] = (mode8, tiles)

            # ---- drain: last unit's AV + O-projection of last block,
            # software-pipelined (AV one sub ahead of its O-projection) with
            # output copies on the otherwise-idle ACT engine ----
            av_sub(NJ - 1, HPG - 1, 0)
            for sub in range(NSUB):
                if sub + 1 < NSUB:
                    av_sub(NJ - 1, HPG - 1, sub + 1)
                oproj_sub(NJ - 1, sub, range(0, 4), copy_engine="mix")

            if debug_dumps:
                nc.sync.dma_start(out=dbg_kt, in_=kt_sb)
                nc.sync.dma_start(out=dbg_qt, in_=qt_sb)
                nc.sync.dma_start(out=dbg_vones, in_=vones)
                nc.sync.dma_start(out=dbg_attn, in_=attn_tiles[(NJ - 1, HPG - 1)][1][0])
                nc.sync.dma_start(out=dbg_aot, in_=aot_tiles[NJ - 1])

    nc.compile()
    return nc


_NC_CACHE = None


def _get_program():
    global _NC_CACHE
    if _NC_CACHE is None:
        _NC_CACHE = build_program()
    return _NC_CACHE


def _hi_lo(a):
    """Split float32 array into fp8e4m3 hi + lo parts."""
    hi = np.asarray(a, dtype=ml_dtypes.float8_e4m3)
    lo = np.asarray(a - hi.astype(np.float32), dtype=ml_dtypes.float8_e4m3)
    return hi, lo


def _mix(a, ncols, slot_order, scale):
    """[D, ncols] f32 -> [128, ND, 2, ncols] fp8 with given hi/lo slot order."""
    a3 = np.ascontiguousarray(a.reshape(ND, 128, ncols)) * np.float32(scale)
    hi, lo = _hi_lo(a3)
    parts = {"hi": hi, "lo": lo}
    stacked = np.stack([parts[slot_order[0]], parts[slot_order[1]]], axis=2)
    return np.ascontiguousarray(stacked.transpose(1, 0, 2, 3))


def _mix_kv(wk_g, wv_g):
    """Two [D, DK] f32 -> [128, ND, 2, 2, DK] fp8 (slot order hi,lo; then
    k/v)."""
    k = _mix(wk_g, DK, ("hi", "lo"), W_SCALE)
    v = _mix(wv_g, DK, ("hi", "lo"), W_SCALE)
    return np.ascontiguousarray(np.stack([k, v], axis=3))


def kernel(x, Wq, bq, Wk, bk, Wv, bv, Wo, bo):
    x = np.asarray(x, np.float32)
    Wq = np.asarray(Wq, np.float32)
    Wk = np.asarray(Wk, np.float32)
    Wv = np.asarray(Wv, np.float32)
    Wo = np.asarray(Wo, np.float32)
    nc = _get_program()

    xmix = [_mix(np.ascontiguousarray(x[b].T), S, ("lo", "hi"), X_SCALE)
            for b in range(x.shape[0])]

    in_maps = []
    for c in range(N_CORES):
        b, g = divmod(c, HPG)
        wo_g = Wo[g * QCOLS:(g + 1) * QCOLS, :].reshape(HPG, 128, D) * np.float32(WO_SCALE)
        wo_hi, wo_lo = _hi_lo(wo_g)
        wo_mix = np.stack([wo_hi, wo_lo], axis=2)  # [HPG, 128, 2, D]
        in_maps.append({
            "xmix": xmix[b],
            "wqmix": _mix(Wq[:, g * QCOLS:(g + 1) * QCOLS], QCOLS, ("hi", "lo"), W_SCALE),
            "wkmix": np.ascontiguousarray(
                _mix(Wk[:, g * DK:(g + 1) * DK], DK, ("hi", "lo"), W_SCALE)
                .transpose(0, 2, 1, 3)),
            "wvmix": np.ascontiguousarray(
                _mix(Wv[:, g * DK:(g + 1) * DK], DK, ("hi", "lo"), W_SCALE)
                .transpose(0, 2, 1, 3)),
            "wo": np.ascontiguousarray(wo_mix.transpose(1, 0, 2, 3)),
            "bq": np.ascontiguousarray(
                np.asarray(bq, np.float32)[g * QCOLS:(g + 1) * QCOLS]
                .reshape(HPG, 128).T),
            "bk": np.ascontiguousarray(
                np.asarray(bk, np.float32)[g * DK:(g + 1) * DK].reshape(128, 1)),
            "bv": np.ascontiguousarray(
                np.asarray(bv, np.float32)[g * DK:(g + 1) * DK].reshape(128, 1)),
        })

    res = run_bass_kernel_spmd(nc, in_maps, core_ids=list(range(N_CORES))).results

    outv = np.zeros((x.shape[0], S, D), np.float32)
    for c in range(N_CORES):
        b = c // HPG
        outv[b] += res[c]["out"]
    outv += np.asarray(bo, np.float32)
    return outv
